# revision 17
# baseline (speedup 1.0000x reference)
"""ConvLSTM block Trainium2 kernel (8 NeuronCores).

Sharding: 8 cores = 4 batches x 2 H-halves. Bottom-half cores process their
slab vertically flipped (with kh-flipped conv kernels) so one SPMD program
serves all cores. Per timestep the two halves of a batch exchange one
boundary row of h via a pairwise AllReduce (halo = sum - own, parity-free).

Per-core compute per step: for each of 2 output-channel tiles (128 ch) and
each chunk of 8 output rows (N=512 pixels), one PSUM accumulation group of
9 matmuls: 3x input conv (K=96: 3 row-taps x 32ch; stride-2 column access
via strided APs) + 6x recurrent conv (K=128: 2 row-taps x 64ch using a
row-shifted duplicate copy of h in partitions 64:128).

Schedule: the halo-consuming boundary chunk runs LAST each step so the
AllReduce latency is covered by the interior chunks; the whole halo path
(DMAs + collective + sub in bf16) lives on the GPSIMD queue so its waits
never block the ACT/DVE FIFOs. Gates are packed {i,o},{f,g} so i and o
activate in one [128,512] ACT op; BN affine and the h dup-copy run on
GPSIMD via tensor_scalar/tensor_copy. Epilogues are emitted split
(partA/partB) one chunk apart so no engine FIFO waits cross-chunk.
"""
import os
import numpy as np

T, H2, W2, F, CIN = 16, 64, 64, 64, 32
WP, XW, NQ = 66, 130, 8
N_HALVES = 2
R = H2 // N_HALVES
SLAB = 2 * R + 1
NCHUNK = R // NQ
MM_DT = os.environ.get("CONV_LSTM_MM_DT", "bf16")  # bf16 | fp32 | fp32r

_CACHE = {}


def _storage_np_dtype():
    import ml_dtypes
    return ml_dtypes.bfloat16 if MM_DT == "bf16" else np.float32


def _prep_core_inputs(x, W, U, b, gamma, beta, moving_mean, moving_var,
                      bidx, half):
    sdt = _storage_np_dtype()
    flip = (half == 1)

    # x slab [T, CIN, SLAB, XW]; XLA SAME (stride2,k3,even) pads bottom/right
    # only: out row r reads input rows 2r..2r+2 (row/col 128 = zero pad).
    xs = np.zeros((T, CIN, SLAB, XW), np.float32)
    xc = np.ascontiguousarray(x[bidx].transpose(0, 3, 1, 2))  # (T,CIN,128,128)
    if not flip:
        xs[:, :, 0:SLAB, 0:128] = xc[:, :, 0:SLAB, :]
    else:
        # slab[s] = x_global[128 - s]; s=0 is the zero pad row
        xs[:, :, 1:SLAB, 0:128] = xc[:, :, 128 - SLAB + 1:][:, :, ::-1, :]

    Wk = W[::-1].copy() if flip else W
    Uk = U[::-1].copy() if flip else U

    # Gate channel packing: m=0 tile holds {o (0:64), i (64:128)},
    # m=1 tile holds {g (0:64), f (64:128)} so i,o share one ACT op and
    # every 2-input DVE op has base-aligned operands (c lives at 64:128).
    sel = [np.r_[192:256, 0:64], np.r_[128:192, 64:128]]

    w3 = np.zeros((96, 768), np.float32)
    ua = np.zeros((128, 768), np.float32)
    ub = np.zeros((128, 768), np.float32)
    # u2: boundary-row variant of ub. Partitions 64:128 carry -U[2] so the
    # kh=2 tap contribution is U2 @ (own+peer) - U2 @ own = U2 @ peer, which
    # lets the AllReduce SUM be DMA'd straight into the h slab (no subtract).
    u2 = np.zeros((128, 768), np.float32)
    for di in range(3):
        for m in range(2):
            g = di * 2 + m
            cols = slice(g * 128, (g + 1) * 128)
            for j in range(3):
                w3[32*j:32*j+32, cols] = Wk[j, di][:, sel[m]]
            ua[0:64, cols] = Uk[0, di][:, sel[m]]
            ua[64:128, cols] = Uk[1, di][:, sel[m]]
            ub[0:64, cols] = Uk[2, di][:, sel[m]]
            u2[0:64, cols] = Uk[2, di][:, sel[m]]
            u2[64:128, cols] = -Uk[2, di][:, sel[m]]

    eps = 1e-3
    scale = (gamma / np.sqrt(moving_var + eps)).astype(np.float32)
    beta2 = (beta - moving_mean * scale).astype(np.float32)
    vecs = np.zeros((128, 8), np.float32)
    # col0: oi ACT bias (hard-sigmoid affine for o rows 0:64, i rows 64:128)
    vecs[0:64, 0] = 0.2 * b[192:256] + 0.5
    vecs[64:128, 0] = 0.2 * b[0:64] + 0.5
    # col1: f ACT bias (input partitions 64:128 of ps1)
    vecs[64:128, 1] = 0.2 * b[64:128] + 0.5
    # col2: g ACT bias (input partitions 0:64 of ps1)
    vecs[0:64, 2] = b[128:192]
    # col3/col4: BN scale / beta for the y affine
    vecs[0:64, 3] = scale
    vecs[0:64, 4] = beta2
    return {
        "xs": np.ascontiguousarray(xs.astype(sdt)),
        "w3": np.ascontiguousarray(w3.astype(sdt)),
        "ua": np.ascontiguousarray(ua.astype(sdt)),
        "ub": np.ascontiguousarray(ub.astype(sdt)),
        "u2": np.ascontiguousarray(u2.astype(sdt)),
        "vecs": vecs,
    }


def _patch_tile_drain():
    """This walrus build encodes at most ONE sync wait per CTRL instruction;
    split the Tile exit drain's waits across SP nops."""
    import bass_rust
    import concourse.tile as tile
    from concourse.vector_clock import ScopedClock
    if getattr(tile.TileContext, "_drain_patched", False):
        return

    def patched(self, tick_clock, wait_clock):
        drain_inst = self.nc.sync.drain()
        wait_clock.add_sem_waits(
            drain_inst.ins, ScopedClock({None: tick_clock.global_clock}))
        si = drain_inst.ins.sync_info
        waits = list(si.on_wait) if si is not None else []
        if len(waits) > 1:
            si.on_wait = waits[:1]
            for w in waits[1:]:
                nop = self.nc.sync.nop()
                nsi = nop.ins.sync_info
                if nsi is None:
                    nop.ins.sync_info = bass_rust.SyncInfo(
                        on_wait=[w], on_update=[])
                else:
                    nsi.on_wait = [w]
        self.nc.all_engine_barrier()
        assert self.sems is not None
        popped = self.nc._tile_sem_poison_stack.pop()
        assert popped is self._sem_poison
        self.nc.clear_and_free_semaphores(list(self.sems.allocated().values()))
        self.nc.all_engine_barrier()

    tile.TileContext._drain_and_barrier = patched
    tile.TileContext._drain_patched = True


def _split_multi_waits(nc, mybir):
    """This walrus build encodes at most one sync wait per instruction;
    move excess waits onto single-wait nops inserted just before."""
    ctr = 0
    for bb in nc.main_func.blocks:
        insts = bb.instructions
        out = []
        changed = False
        for inst in insts:
            si = inst.sync_info
            waits = list(si.on_wait) if si is not None else []
            if len(waits) > 1:
                changed = True
                for w in waits[:-1]:
                    ctr += 1
                    out.append(mybir.InstNoOp(
                        name=f"wsplit-{ctr}",
                        engine=inst.engine,
                        sync_info=mybir.SyncInfo(on_wait=[w], on_update=[]),
                        bass_nofuse=True))
                si.on_wait = [waits[-1]]
            out.append(inst)
        if changed:
            bb.instructions = out


def _build_nc():
    import concourse.bass as bass
    import concourse.mybir as mybir
    import concourse.tile as tile
    _patch_tile_drain()
    dt = mybir.dt
    sdt = dt.bfloat16 if MM_DT == "bf16" else dt.float32
    AF = mybir.ActivationFunctionType

    def mm_ap(ap):
        return ap.bitcast(dt.float32r) if MM_DT == "fp32r" else ap

    nc = bass.Bass()
    xs = nc.dram_tensor("xs", [T, CIN, SLAB, XW], sdt, kind="ExternalInput")
    w3 = nc.dram_tensor("w3", [96, 768], sdt, kind="ExternalInput")
    ua = nc.dram_tensor("ua", [128, 768], sdt, kind="ExternalInput")
    ub = nc.dram_tensor("ub", [128, 768], sdt, kind="ExternalInput")
    u2 = nc.dram_tensor("u2", [128, 768], sdt, kind="ExternalInput")
    vecs = nc.dram_tensor("vecs", [128, 8], dt.float32, kind="ExternalInput")
    y = nc.dram_tensor("y", [T, F, R * W2], dt.float32, kind="ExternalOutput")

    groups = [[0, 1], [2, 3], [4, 5], [6, 7]]

    with tile.TileContext(nc) as tc:
        with (
            tc.tile_pool(name="const", bufs=1) as cpool,
            tc.tile_pool(name="state", bufs=1) as spool,
            tc.tile_pool(name="xp", bufs=2) as xpool,
            tc.tile_pool(name="ps", bufs=6, space="PSUM") as pspool,
            tc.tile_pool(name="psb", bufs=2, space="PSUM") as psbpool,
            tc.tile_pool(name="epi", bufs=3) as epool,
            tc.tile_pool(name="yp", bufs=4) as ypool,
            tc.tile_pool(name="halo", bufs=2) as hpool,
            tc.tile_pool(name="dram", bufs=2, space="DRAM") as dpool,
        ):
            w3sb = cpool.tile([96, 768], sdt, tag="w3sb")
            uasb = cpool.tile([128, 768], sdt, tag="uasb")
            ubsb = cpool.tile([128, 768], sdt, tag="ubsb")
            u2sb = cpool.tile([128, 768], sdt, tag="u2sb")
            vsb = cpool.tile([128, 8], dt.float32, tag="vsb")
            nc.sync.dma_start(out=w3sb[:], in_=w3[:])
            nc.sync.dma_start(out=uasb[:], in_=ua[:])
            nc.sync.dma_start(out=ubsb[:], in_=ub[:])
            nc.sync.dma_start(out=u2sb[:], in_=u2[:])
            nc.sync.dma_start(out=vsb[:], in_=vecs[:])

            h2 = [spool.tile([128, (R + 2) * WP], sdt, name=f"h2_{i}",
                             tag=f"h2_{i}")
                  for i in range(2)]
            # cell state lives at partitions 64:128 so t1/t2/add are
            # base-aligned with f/i/g (which come out of PSUM upper halves)
            c_sb = spool.tile([128, R * W2], dt.float32, tag="c")
            nc.vector.memset(h2[0][:], 0.0)
            nc.vector.memset(h2[1][:], 0.0)
            nc.vector.memset(c_sb[64:128, :], 0.0)

            def emit_mms(t, q0, nq, hpr, x3r):
                # boundary single-row chunk: separate PSUM pool, u2 weights
                # ([U2; -U2] reading [sum; own]), and m-interleaved order so
                # the only halo-dependent MMs are the 6 tiny trailing ones
                bnd = (nq == 1)
                pool, wtap2 = (psbpool, u2sb) if bnd else (pspool, ubsb)
                n = nq * W2
                pss = [pool.tile([128, n], dt.float32,
                                 name=f"ps_{t}_{q0}_{mi}",
                                 tag="psb" if bnd else "ps")
                       for mi in range(2)]
                order = ([(m, s) for s in (0, 1) for m in (0, 1)]
                         + [(m, 2) for m in (0, 1)]) if bnd else \
                        [(m, s) for m in (0, 1) for s in (0, 1, 2)]
                nmm = {m: 0 for m in (0, 1)}
                for m, stage in order:
                    psr = pss[m][:].rearrange("p (a b) -> p a b", b=W2)
                    for di in range(3):
                        d = di - 1
                        gcol = slice((di*2+m)*128, (di*2+m+1)*128)
                        if stage == 0:
                            lhsT, rhs = w3sb[0:96, gcol], \
                                x3r[0:96, q0:q0+nq, d+1:d+129:2]
                        elif stage == 1:
                            lhsT, rhs = uasb[0:128, gcol], \
                                hpr[0:128, q0:q0+nq, 1+d:65+d]
                        else:
                            lhsT, rhs = wtap2[0:128, gcol], \
                                hpr[0:128, q0+2:q0+nq+2, 1+d:65+d]
                        nc.tensor.matmul(
                            psr[:], lhsT=mm_ap(lhsT), rhs=mm_ap(rhs),
                            start=(nmm[m] == 0), stop=(nmm[m] == 8))
                        nmm[m] += 1
                return pss

            def emit_partA(t, q0, nq, pss):
                ps0, ps1 = pss
                n = nq * W2
                sfx = "b" if nq == 1 else ""
                cs = slice(q0 * W2, (q0 + nq) * W2)
                # oi: one [128,n] hard-sigmoid-affine ACT (o at 0:64, i at
                # 64:128; clip via min in the gate products below)
                io_t = epool.tile([128, n], dt.float32, tag="io" + sfx)
                nc.scalar.activation(io_t[:], ps0[:], AF.Relu,
                                     bias=vsb[:, 0:1], scale=0.2)
                f_t = epool.tile([128, n], dt.float32, tag="f" + sfx)
                nc.scalar.activation(f_t[64:128, :], ps1[64:128, :], AF.Relu,
                                     bias=vsb[64:128, 1:2], scale=0.2)
                g_t = epool.tile([128, n], dt.float32, tag="g" + sfx)
                nc.scalar.activation(g_t[64:128, :], ps1[0:64, :], AF.Tanh,
                                     bias=vsb[0:64, 2:3], scale=1.0)
                t1 = epool.tile([128, n], dt.float32, tag="t1" + sfx)
                nc.vector.scalar_tensor_tensor(
                    t1[64:128, :], f_t[64:128, :], 1.0, c_sb[64:128, cs],
                    mybir.AluOpType.min, mybir.AluOpType.mult)
                t2 = epool.tile([128, n], dt.float32, tag="t2" + sfx)
                nc.vector.scalar_tensor_tensor(
                    t2[64:128, :], io_t[64:128, :], 1.0, g_t[64:128, :],
                    mybir.AluOpType.min, mybir.AluOpType.mult)
                nc.vector.tensor_add(c_sb[64:128, cs], t1[64:128, :],
                                     t2[64:128, :])
                return io_t

            def emit_partB(t, q0, nq, io_t, hcr):
                n = nq * W2
                sfx = "b" if nq == 1 else ""
                cs = slice(q0 * W2, (q0 + nq) * W2)
                tc_t = epool.tile([64, n], dt.float32, tag="tc" + sfx)
                nc.scalar.activation(tc_t[:], c_sb[64:128, cs], AF.Tanh)
                hlo = hcr[0:64, q0+1:q0+nq+1, 1:65]
                nc.vector.scalar_tensor_tensor(
                    hlo,
                    io_t[0:64, :].rearrange("p (a b) -> p a b", b=W2), 1.0,
                    tc_t[:].rearrange("p (a b) -> p a b", b=W2),
                    mybir.AluOpType.min, mybir.AluOpType.mult)
                nc.vector.tensor_copy(
                    out=hcr[64:128, q0:q0+nq, 1:65], in_=hlo)
                if nq == 1:
                    # own h[R-1] duplicated at q=R+1 parts 64:128: pairs with
                    # -U2 against the AllReduce sum landing at q=R+1 parts 0:64
                    nc.vector.tensor_copy(
                        out=hcr[64:128, R+1, 1:65], in_=hlo)
                yst = ypool.tile([64, n], dt.float32, tag="yst" + sfx)
                yeng = nc.vector if nq == 1 else nc.gpsimd
                yeng.tensor_scalar(
                    yst[:].rearrange("p (a b) -> p a b", b=W2), hlo,
                    vsb[0:64, 3:4], vsb[0:64, 4:5],
                    mybir.AluOpType.mult, mybir.AluOpType.add)
                nc.sync.dma_start(out=y[t, :, q0*W2:(q0+nq)*W2], in_=yst[:])

            for t in range(T):
                hc = h2[t % 2]
                hp = h2[(t + 1) % 2]
                hcr = hc[:].rearrange("p (q w) -> p q w", w=WP)
                hpr = hp[:].rearrange("p (q w) -> p q w", w=WP)

                x3t = xpool.tile([96, R * XW], sdt, tag="x3")
                x3r = x3t[:].rearrange("p (q w) -> p q w", w=XW)
                nc.sync.dma_start(out=x3r[0:32], in_=xs[t, :, 0:2*R-1:2, :])
                nc.sync.dma_start(out=x3r[32:64], in_=xs[t, :, 1:2*R:2, :])
                nc.sync.dma_start(out=x3r[64:96], in_=xs[t, :, 2:2*R+1:2, :])

                # boundary single-row chunk FIRST: its halo input (bout of
                # step t-1) launched at the top of step t-1, so a full step
                # of interior work covers the mesh latency; likewise the
                # exchange launched here is consumed a full step downstream.
                pss = emit_mms(t, R - 1, 1, hpr, x3r)
                io_t = emit_partA(t, R - 1, 1, pss)
                emit_partB(t, R - 1, 1, io_t, hcr)

                # halo exchange (skip after last step): bf16; the AllReduce
                # SUM lands directly at q=R+1 parts 0:64 (u2's -U2 rows
                # subtract own inside the boundary matmul). DMAs + trigger on
                # the GPSIMD queue so waits never block ACT/DVE/PE.
                if t < T - 1:
                    bin_d = dpool.tile([64, 64], sdt, tag="bin")
                    bout_d = dpool.tile([64, 64], sdt, tag="bout")
                    nc.gpsimd.dma_start(out=bin_d[:],
                                        in_=hcr[0:64, R, 1:65])
                    nc.gpsimd.collective_compute(
                        "AllReduce", mybir.AluOpType.add,
                        replica_groups=groups,
                        ins=[bin_d[:].opt()], outs=[bout_d[:].opt()])
                    nc.gpsimd.dma_start(out=hcr[0:64, R+1, 1:65],
                                        in_=bout_d[:])

                # interior chunks; partB emitted one chunk behind partA so
                # no engine FIFO waits cross-chunk.
                chunks = [(0, NQ), (NQ, NQ), (2 * NQ, NQ), (3 * NQ, NQ - 1)]
                prevB = None
                for q0, nq in chunks:
                    pss = emit_mms(t, q0, nq, hpr, x3r)
                    io_t = emit_partA(t, q0, nq, pss)
                    if prevB is not None:
                        emit_partB(t, prevB[0], prevB[1], prevB[2], hcr)
                    prevB = (q0, nq, io_t)
                emit_partB(t, prevB[0], prevB[1], prevB[2], hcr)
    _split_multi_waits(nc, mybir)
    return nc


def _install_ntff_hook():
    """The image's antenv lacks axon_hooks; synthesize it and register the
    ctypes NTFF profile hook so trace=True works under axon."""
    import sys
    import types
    try:
        from antenv.axon_hooks import get_axon_ntff_profile_hook  # noqa
        return
    except ImportError:
        pass
    mod = types.ModuleType("antenv.axon_hooks")
    mod._hook = None

    def set_axon_ntff_profile_hook(h):
        mod._hook = h

    def get_axon_ntff_profile_hook():
        return mod._hook

    mod.set_axon_ntff_profile_hook = set_axon_ntff_profile_hook
    mod.get_axon_ntff_profile_hook = get_axon_ntff_profile_hook
    sys.modules["antenv.axon_hooks"] = mod
    import antenv
    antenv.axon_hooks = mod
    try:
        from trn_agent_boot.trn_boot import _ntff_profile_via_ctypes
        hook = _ntff_profile_via_ctypes("/opt/axon/libaxon_pjrt.so")
        if hook is not None:
            mod._hook = hook
    except Exception:
        pass


def _get_nc():
    key = (MM_DT,)
    if key not in _CACHE:
        _CACHE[key] = _build_nc()
    return _CACHE[key]


def kernel(x, W, U, b, gamma, beta, moving_mean, moving_var):
    from concourse.bass_utils import run_bass_kernel_spmd
    x = np.asarray(x, np.float32)
    W = np.asarray(W, np.float32)
    U = np.asarray(U, np.float32)
    b = np.asarray(b, np.float32)
    gamma = np.asarray(gamma, np.float32)
    beta = np.asarray(beta, np.float32)
    moving_mean = np.asarray(moving_mean, np.float32)
    moving_var = np.asarray(moving_var, np.float32)
    B = x.shape[0]

    in_maps = []
    for bidx in range(B):
        for half in range(N_HALVES):
            in_maps.append(_prep_core_inputs(
                x, W, U, b, gamma, beta, moving_mean, moving_var, bidx, half))

    nc = _get_nc()
    trace = os.environ.get("BASS_KERNEL_TRACE") == "1"
    if trace:
        _install_ntff_hook()
    res = run_bass_kernel_spmd(nc, in_maps, core_ids=list(range(8)),
                               trace=trace)
    kernel._last_result = res

    out = np.zeros((B, T, H2, W2, F), np.float32)
    ci = 0
    for bidx in range(B):
        for half in range(N_HALVES):
            yc = res.results[ci]["y"].reshape(T, F, R, W2)
            ci += 1
            yc = yc.transpose(0, 2, 3, 1)  # (T, R, W2, F)
            if half == 1:
                yc = yc[:, ::-1, :, :]
                out[bidx, :, 32:64] = yc
            else:
                out[bidx, :, 0:32] = yc
    return out


# revision 23
# speedup vs baseline: 1.0025x; 1.0025x over previous
"""ConvLSTM block Trainium2 kernel (8 NeuronCores).

Sharding: 8 cores = 4 batches x 2 H-halves. Bottom-half cores process their
slab vertically flipped (with kh-flipped conv kernels) so one SPMD program
serves all cores. Per timestep the two halves of a batch exchange one
boundary row of h via a pairwise AllReduce (halo = sum - own, parity-free).

Per-core compute per step: for each of 2 output-channel tiles (128 ch) and
each chunk of 8 output rows (N=512 pixels), one PSUM accumulation group of
9 matmuls: 3x input conv (K=96: 3 row-taps x 32ch; stride-2 column access
via strided APs) + 6x recurrent conv (K=128: 2 row-taps x 64ch using a
row-shifted duplicate copy of h in partitions 64:128).

Schedule: the halo-consuming boundary chunk runs LAST each step so the
AllReduce latency is covered by the interior chunks; the whole halo path
(DMAs + collective + sub in bf16) lives on the GPSIMD queue so its waits
never block the ACT/DVE FIFOs. Gates are packed {i,o},{f,g} so i and o
activate in one [128,512] ACT op; BN affine and the h dup-copy run on
GPSIMD via tensor_scalar/tensor_copy. Epilogues are emitted split
(partA/partB) one chunk apart so no engine FIFO waits cross-chunk.
"""
import os
import numpy as np

T, H2, W2, F, CIN = 16, 64, 64, 64, 32
WP, XW, NQ = 66, 130, 8
N_HALVES = 2
R = H2 // N_HALVES
SLAB = 2 * R + 1
NCHUNK = R // NQ
MM_DT = os.environ.get("CONV_LSTM_MM_DT", "bf16")  # bf16 | fp32 | fp32r

_CACHE = {}


def _storage_np_dtype():
    import ml_dtypes
    return ml_dtypes.bfloat16 if MM_DT == "bf16" else np.float32


def _prep_core_inputs(x, W, U, b, gamma, beta, moving_mean, moving_var,
                      bidx, half):
    sdt = _storage_np_dtype()
    flip = (half == 1)

    # x slab [T, CIN, SLAB, XW]; XLA SAME (stride2,k3,even) pads bottom/right
    # only: out row r reads input rows 2r..2r+2 (row/col 128 = zero pad).
    xs = np.zeros((T, CIN, SLAB, XW), np.float32)
    xc = np.ascontiguousarray(x[bidx].transpose(0, 3, 1, 2))  # (T,CIN,128,128)
    if not flip:
        xs[:, :, 0:SLAB, 0:128] = xc[:, :, 0:SLAB, :]
    else:
        # slab[s] = x_global[128 - s]; s=0 is the zero pad row
        xs[:, :, 1:SLAB, 0:128] = xc[:, :, 128 - SLAB + 1:][:, :, ::-1, :]

    Wk = W[::-1].copy() if flip else W
    Uk = U[::-1].copy() if flip else U

    # Gate channel packing: m=0 tile holds {o (0:64), i (64:128)},
    # m=1 tile holds {g (0:64), f (64:128)} so i,o share one ACT op and
    # every 2-input DVE op has base-aligned operands (c lives at 64:128).
    sel = [np.r_[192:256, 0:64], np.r_[128:192, 64:128]]

    w3 = np.zeros((96, 768), np.float32)
    ua = np.zeros((128, 768), np.float32)
    ub = np.zeros((128, 768), np.float32)
    # u2 rows 0:64 = U[1]-U[2]: the boundary row reads the AllReduce SUM
    # (own+peer) through U[2] at q=R+1, so its own-h31 tap needs U[1]-U[2]
    # (U2@(own+peer) - U2@own = U2@peer). No subtract, no dup copies.
    u2 = np.zeros((128, 768), np.float32)
    for di in range(3):
        for m in range(2):
            g = di * 2 + m
            cols = slice(g * 128, (g + 1) * 128)
            for j in range(3):
                w3[32*j:32*j+32, cols] = Wk[j, di][:, sel[m]]
            ua[0:64, cols] = Uk[0, di][:, sel[m]]
            ua[64:128, cols] = Uk[1, di][:, sel[m]]
            ub[0:64, cols] = Uk[2, di][:, sel[m]]
            u2[0:64, cols] = (Uk[1, di] - Uk[2, di])[:, sel[m]]

    eps = 1e-3
    scale = (gamma / np.sqrt(moving_var + eps)).astype(np.float32)
    beta2 = (beta - moving_mean * scale).astype(np.float32)
    vecs = np.zeros((128, 8), np.float32)
    # col0: oi ACT bias (hard-sigmoid affine for o rows 0:64, i rows 64:128)
    vecs[0:64, 0] = 0.2 * b[192:256] + 0.5
    vecs[64:128, 0] = 0.2 * b[0:64] + 0.5
    # col1: f ACT bias (input partitions 64:128 of ps1)
    vecs[64:128, 1] = 0.2 * b[64:128] + 0.5
    # col2: g ACT bias (input partitions 0:64 of ps1)
    vecs[0:64, 2] = b[128:192]
    # col3/col4: BN scale / beta for the y affine
    vecs[0:64, 3] = scale
    vecs[0:64, 4] = beta2
    return {
        "xs": np.ascontiguousarray(xs.astype(sdt)),
        "w3": np.ascontiguousarray(w3.astype(sdt)),
        "ua": np.ascontiguousarray(ua.astype(sdt)),
        "ub": np.ascontiguousarray(ub.astype(sdt)),
        "u2": np.ascontiguousarray(u2.astype(sdt)),
        "vecs": vecs,
    }


def _patch_tile_drain():
    """This walrus build encodes at most ONE sync wait per CTRL instruction;
    split the Tile exit drain's waits across SP nops."""
    import bass_rust
    import concourse.tile as tile
    from concourse.vector_clock import ScopedClock
    if getattr(tile.TileContext, "_drain_patched", False):
        return

    def patched(self, tick_clock, wait_clock):
        drain_inst = self.nc.sync.drain()
        wait_clock.add_sem_waits(
            drain_inst.ins, ScopedClock({None: tick_clock.global_clock}))
        si = drain_inst.ins.sync_info
        waits = list(si.on_wait) if si is not None else []
        if len(waits) > 1:
            si.on_wait = waits[:1]
            for w in waits[1:]:
                nop = self.nc.sync.nop()
                nsi = nop.ins.sync_info
                if nsi is None:
                    nop.ins.sync_info = bass_rust.SyncInfo(
                        on_wait=[w], on_update=[])
                else:
                    nsi.on_wait = [w]
        self.nc.all_engine_barrier()
        assert self.sems is not None
        popped = self.nc._tile_sem_poison_stack.pop()
        assert popped is self._sem_poison
        self.nc.clear_and_free_semaphores(list(self.sems.allocated().values()))
        self.nc.all_engine_barrier()

    tile.TileContext._drain_and_barrier = patched
    tile.TileContext._drain_patched = True


def _split_multi_waits(nc, mybir):
    """This walrus build encodes at most one sync wait per instruction;
    move excess waits onto single-wait nops inserted just before."""
    ctr = 0
    for bb in nc.main_func.blocks:
        insts = bb.instructions
        out = []
        changed = False
        for inst in insts:
            si = inst.sync_info
            waits = list(si.on_wait) if si is not None else []
            if len(waits) > 1:
                changed = True
                for w in waits[:-1]:
                    ctr += 1
                    out.append(mybir.InstNoOp(
                        name=f"wsplit-{ctr}",
                        engine=inst.engine,
                        sync_info=mybir.SyncInfo(on_wait=[w], on_update=[]),
                        bass_nofuse=True))
                si.on_wait = [waits[-1]]
            out.append(inst)
        if changed:
            bb.instructions = out


def _build_nc():
    import concourse.bass as bass
    import concourse.mybir as mybir
    import concourse.tile as tile
    _patch_tile_drain()
    dt = mybir.dt
    sdt = dt.bfloat16 if MM_DT == "bf16" else dt.float32
    AF = mybir.ActivationFunctionType

    def mm_ap(ap):
        return ap.bitcast(dt.float32r) if MM_DT == "fp32r" else ap

    nc = bass.Bass()
    xs = nc.dram_tensor("xs", [T, CIN, SLAB, XW], sdt, kind="ExternalInput")
    w3 = nc.dram_tensor("w3", [96, 768], sdt, kind="ExternalInput")
    ua = nc.dram_tensor("ua", [128, 768], sdt, kind="ExternalInput")
    ub = nc.dram_tensor("ub", [128, 768], sdt, kind="ExternalInput")
    u2 = nc.dram_tensor("u2", [128, 768], sdt, kind="ExternalInput")
    vecs = nc.dram_tensor("vecs", [128, 8], dt.float32, kind="ExternalInput")
    y = nc.dram_tensor("y", [T, F, R * W2], dt.float32, kind="ExternalOutput")

    groups = [[0, 1], [2, 3], [4, 5], [6, 7]]

    with tile.TileContext(nc) as tc:
        with (
            tc.tile_pool(name="const", bufs=1) as cpool,
            tc.tile_pool(name="state", bufs=1) as spool,
            tc.tile_pool(name="xp", bufs=2) as xpool,
            tc.tile_pool(name="ps", bufs=6, space="PSUM") as pspool,
            tc.tile_pool(name="psb", bufs=2, space="PSUM") as psbpool,
            tc.tile_pool(name="epi", bufs=3) as epool,
            tc.tile_pool(name="yp", bufs=4) as ypool,
            tc.tile_pool(name="halo", bufs=2) as hpool,
            tc.tile_pool(name="dram", bufs=2, space="DRAM") as dpool,
        ):
            w3sb = cpool.tile([96, 768], sdt, tag="w3sb")
            uasb = cpool.tile([128, 768], sdt, tag="uasb")
            ubsb = cpool.tile([128, 768], sdt, tag="ubsb")
            u2sb = cpool.tile([128, 768], sdt, tag="u2sb")
            vsb = cpool.tile([128, 8], dt.float32, tag="vsb")
            nc.sync.dma_start(out=w3sb[:], in_=w3[:])
            nc.sync.dma_start(out=uasb[:], in_=ua[:])
            nc.sync.dma_start(out=ubsb[:], in_=ub[:])
            nc.sync.dma_start(out=u2sb[:], in_=u2[:])
            nc.sync.dma_start(out=vsb[:], in_=vecs[:])

            h2 = [spool.tile([128, (R + 2) * WP], sdt, name=f"h2_{i}",
                             tag=f"h2_{i}")
                  for i in range(2)]
            # cell state lives at partitions 64:128 so t1/t2/add are
            # base-aligned with f/i/g (which come out of PSUM upper halves)
            c_sb = spool.tile([128, R * W2], dt.float32, tag="c")
            nc.vector.memset(h2[0][:], 0.0)
            nc.vector.memset(h2[1][:], 0.0)
            nc.vector.memset(c_sb[64:128, :], 0.0)

            def emit_mms(t, q0, nq, hpr, x3r):
                bnd = (nq == 1)
                pool = psbpool if bnd else pspool
                n = nq * W2
                pss = [pool.tile([128, n], dt.float32,
                                 name=f"ps_{t}_{q0}_{mi}",
                                 tag="psb" if bnd else "ps")
                       for mi in range(2)]
                if bnd:
                    # 12 K<=96 MMs: w3 (x), ua0 = U0@h30, uc = (U1-U2)@h31
                    # (reads hlo of the previous boundary row directly),
                    # u2a = U2@SUM last (the only halo-dependent MMs)
                    stages = [
                        (w3sb, slice(0, 96),
                         lambda d: x3r[0:96, q0:q0+1, d+1:d+129:2]),
                        (uasb, slice(0, 64),
                         lambda d: hpr[0:64, q0:q0+1, 1+d:65+d]),
                        (u2sb, slice(0, 64),
                         lambda d: hpr[0:64, q0+1:q0+2, 1+d:65+d]),
                        (ubsb, slice(0, 64),
                         lambda d: hpr[0:64, q0+2:q0+3, 1+d:65+d]),
                    ]
                    order = [(m, s) for s in range(4) for m in (0, 1)]
                    last = 11
                else:
                    stages = [
                        (w3sb, slice(0, 96),
                         lambda d: x3r[0:96, q0:q0+nq, d+1:d+129:2]),
                        (uasb, slice(0, 128),
                         lambda d: hpr[0:128, q0:q0+nq, 1+d:65+d]),
                        (ubsb, slice(0, 128),
                         lambda d: hpr[0:128, q0+2:q0+nq+2, 1+d:65+d]),
                    ]
                    order = [(m, s) for m in (0, 1) for s in range(3)]
                    last = 8
                nmm = {m: 0 for m in (0, 1)}
                for m, stage in order:
                    psr = pss[m][:].rearrange("p (a b) -> p a b", b=W2)
                    wsb, rows, rhs_fn = stages[stage]
                    for di in range(3):
                        d = di - 1
                        gcol = slice((di*2+m)*128, (di*2+m+1)*128)
                        nc.tensor.matmul(
                            psr[:], lhsT=mm_ap(wsb[rows, gcol]),
                            rhs=mm_ap(rhs_fn(d)),
                            start=(nmm[m] == 0), stop=(nmm[m] == last))
                        nmm[m] += 1
                return pss

            def emit_partA(t, q0, nq, pss):
                ps0, ps1 = pss
                n = nq * W2
                sfx = "b" if nq == 1 else ""
                cs = slice(q0 * W2, (q0 + nq) * W2)
                # oi: one [128,n] hard-sigmoid-affine ACT (o at 0:64, i at
                # 64:128; clip via min in the gate products below)
                io_t = epool.tile([128, n], dt.float32, tag="io" + sfx)
                nc.scalar.activation(io_t[:], ps0[:], AF.Relu,
                                     bias=vsb[:, 0:1], scale=0.2)
                f_t = epool.tile([128, n], dt.float32, tag="f" + sfx)
                nc.scalar.activation(f_t[64:128, :], ps1[64:128, :], AF.Relu,
                                     bias=vsb[64:128, 1:2], scale=0.2)
                g_t = epool.tile([128, n], dt.float32, tag="g" + sfx)
                nc.scalar.activation(g_t[64:128, :], ps1[0:64, :], AF.Tanh,
                                     bias=vsb[0:64, 2:3], scale=1.0)
                t1 = epool.tile([128, n], dt.float32, tag="t1" + sfx)
                nc.vector.scalar_tensor_tensor(
                    t1[64:128, :], f_t[64:128, :], 1.0, c_sb[64:128, cs],
                    mybir.AluOpType.min, mybir.AluOpType.mult)
                t2 = epool.tile([128, n], dt.float32, tag="t2" + sfx)
                nc.vector.scalar_tensor_tensor(
                    t2[64:128, :], io_t[64:128, :], 1.0, g_t[64:128, :],
                    mybir.AluOpType.min, mybir.AluOpType.mult)
                nc.vector.tensor_add(c_sb[64:128, cs], t1[64:128, :],
                                     t2[64:128, :])
                return io_t

            def emit_partB(t, q0, nq, io_t, hcr):
                n = nq * W2
                sfx = "b" if nq == 1 else ""
                cs = slice(q0 * W2, (q0 + nq) * W2)
                tc_t = epool.tile([64, n], dt.float32, tag="tc" + sfx)
                nc.scalar.activation(tc_t[:], c_sb[64:128, cs], AF.Tanh)
                hlo = hcr[0:64, q0+1:q0+nq+1, 1:65]
                nc.vector.scalar_tensor_tensor(
                    hlo,
                    io_t[0:64, :].rearrange("p (a b) -> p a b", b=W2), 1.0,
                    tc_t[:].rearrange("p (a b) -> p a b", b=W2),
                    mybir.AluOpType.min, mybir.AluOpType.mult)
                if nq > 1:
                    nc.vector.tensor_copy(
                        out=hcr[64:128, q0:q0+nq, 1:65], in_=hlo)
                yst = ypool.tile([64, n], dt.float32, tag="yst" + sfx)
                yeng = nc.vector if nq == 1 else nc.gpsimd
                yeng.tensor_scalar(
                    yst[:].rearrange("p (a b) -> p a b", b=W2), hlo,
                    vsb[0:64, 3:4], vsb[0:64, 4:5],
                    mybir.AluOpType.mult, mybir.AluOpType.add)
                nc.sync.dma_start(out=y[t, :, q0*W2:(q0+nq)*W2], in_=yst[:])

            for t in range(T):
                hc = h2[t % 2]
                hp = h2[(t + 1) % 2]
                hcr = hc[:].rearrange("p (q w) -> p q w", w=WP)
                hpr = hp[:].rearrange("p (q w) -> p q w", w=WP)

                x3t = xpool.tile([96, R * XW], sdt, tag="x3")
                x3r = x3t[:].rearrange("p (q w) -> p q w", w=XW)
                nc.sync.dma_start(out=x3r[0:32], in_=xs[t, :, 0:2*R-1:2, :])
                nc.sync.dma_start(out=x3r[32:64], in_=xs[t, :, 1:2*R:2, :])
                nc.sync.dma_start(out=x3r[64:96], in_=xs[t, :, 2:2*R+1:2, :])

                # boundary single-row chunk FIRST: its halo input (bout of
                # step t-1) launched at the top of step t-1, so a full step
                # of interior work covers the mesh latency; likewise the
                # exchange launched here is consumed a full step downstream.
                pss = emit_mms(t, R - 1, 1, hpr, x3r)
                io_t = emit_partA(t, R - 1, 1, pss)
                emit_partB(t, R - 1, 1, io_t, hcr)

                # halo exchange (skip after last step): bf16; the AllReduce
                # SUM lands directly at q=R+1 parts 0:64 (u2's -U2 rows
                # subtract own inside the boundary matmul). DMAs + trigger on
                # the GPSIMD queue so waits never block ACT/DVE/PE.
                if t < T - 1:
                    bin_d = dpool.tile([64, 64], sdt, tag="bin")
                    bout_d = dpool.tile([64, 64], sdt, tag="bout")
                    nc.gpsimd.dma_start(out=bin_d[:],
                                        in_=hcr[0:64, R, 1:65])
                    nc.gpsimd.collective_compute(
                        "AllReduce", mybir.AluOpType.add,
                        replica_groups=groups,
                        ins=[bin_d[:].opt()], outs=[bout_d[:].opt()])
                    nc.gpsimd.dma_start(out=hcr[0:64, R+1, 1:65],
                                        in_=bout_d[:])

                # interior chunks (c3a third so the boundary row's h[30]
                # input lands mid-step, and every cross-step h edge has
                # >= half a step of slack); partB emitted one chunk behind
                # partA so no engine FIFO waits cross-chunk.
                chunks = [(0, NQ), (NQ, NQ), (3 * NQ, NQ - 1), (2 * NQ, NQ)]
                prevB = None
                for q0, nq in chunks:
                    pss = emit_mms(t, q0, nq, hpr, x3r)
                    io_t = emit_partA(t, q0, nq, pss)
                    if prevB is not None:
                        emit_partB(t, prevB[0], prevB[1], prevB[2], hcr)
                    prevB = (q0, nq, io_t)
                emit_partB(t, prevB[0], prevB[1], prevB[2], hcr)
    _split_multi_waits(nc, mybir)
    return nc


def _install_ntff_hook():
    """The image's antenv lacks axon_hooks; synthesize it and register the
    ctypes NTFF profile hook so trace=True works under axon."""
    import sys
    import types
    try:
        from antenv.axon_hooks import get_axon_ntff_profile_hook  # noqa
        return
    except ImportError:
        pass
    mod = types.ModuleType("antenv.axon_hooks")
    mod._hook = None

    def set_axon_ntff_profile_hook(h):
        mod._hook = h

    def get_axon_ntff_profile_hook():
        return mod._hook

    mod.set_axon_ntff_profile_hook = set_axon_ntff_profile_hook
    mod.get_axon_ntff_profile_hook = get_axon_ntff_profile_hook
    sys.modules["antenv.axon_hooks"] = mod
    import antenv
    antenv.axon_hooks = mod
    try:
        from trn_agent_boot.trn_boot import _ntff_profile_via_ctypes
        hook = _ntff_profile_via_ctypes("/opt/axon/libaxon_pjrt.so")
        if hook is not None:
            mod._hook = hook
    except Exception:
        pass


def _get_nc():
    key = (MM_DT,)
    if key not in _CACHE:
        _CACHE[key] = _build_nc()
    return _CACHE[key]


def kernel(x, W, U, b, gamma, beta, moving_mean, moving_var):
    from concourse.bass_utils import run_bass_kernel_spmd
    x = np.asarray(x, np.float32)
    W = np.asarray(W, np.float32)
    U = np.asarray(U, np.float32)
    b = np.asarray(b, np.float32)
    gamma = np.asarray(gamma, np.float32)
    beta = np.asarray(beta, np.float32)
    moving_mean = np.asarray(moving_mean, np.float32)
    moving_var = np.asarray(moving_var, np.float32)
    B = x.shape[0]

    in_maps = []
    for bidx in range(B):
        for half in range(N_HALVES):
            in_maps.append(_prep_core_inputs(
                x, W, U, b, gamma, beta, moving_mean, moving_var, bidx, half))

    nc = _get_nc()
    trace = os.environ.get("BASS_KERNEL_TRACE") == "1"
    if trace:
        _install_ntff_hook()
    res = run_bass_kernel_spmd(nc, in_maps, core_ids=list(range(8)),
                               trace=trace)
    kernel._last_result = res

    out = np.zeros((B, T, H2, W2, F), np.float32)
    ci = 0
    for bidx in range(B):
        for half in range(N_HALVES):
            yc = res.results[ci]["y"].reshape(T, F, R, W2)
            ci += 1
            yc = yc.transpose(0, 2, 3, 1)  # (T, R, W2, F)
            if half == 1:
                yc = yc[:, ::-1, :, :]
                out[bidx, :, 32:64] = yc
            else:
                out[bidx, :, 0:32] = yc
    return out


# revision 31
# speedup vs baseline: 1.0467x; 1.0442x over previous
"""ConvLSTM block Trainium2 kernel (8 NeuronCores).

Sharding: 8 cores = 4 batches x 2 H-halves. Bottom-half cores process their
slab vertically flipped (with kh-flipped conv kernels) so one SPMD program
serves all cores. Per timestep the two halves of a batch exchange one
boundary row of h via a pairwise AllReduce (halo = sum - own, parity-free).

Per-core compute per step: for each of 2 output-channel tiles (128 ch) and
each chunk of 8 output rows (N=512 pixels), one PSUM accumulation group of
9 matmuls: 3x input conv (K=96: 3 row-taps x 32ch; stride-2 column access
via strided APs) + 6x recurrent conv (K=128: 2 row-taps x 64ch using a
row-shifted duplicate copy of h in partitions 64:128).

Schedule: the halo-consuming boundary chunk runs LAST each step so the
AllReduce latency is covered by the interior chunks; the whole halo path
(DMAs + collective + sub in bf16) lives on the GPSIMD queue so its waits
never block the ACT/DVE FIFOs. Gates are packed {i,o},{f,g} so i and o
activate in one [128,512] ACT op; BN affine and the h dup-copy run on
GPSIMD via tensor_scalar/tensor_copy. Epilogues are emitted split
(partA/partB) one chunk apart so no engine FIFO waits cross-chunk.
"""
import os
import numpy as np

T, H2, W2, F, CIN = 16, 64, 64, 64, 32
WP, XW, NQ = 66, 130, 8
N_HALVES = 2
R = H2 // N_HALVES
SLAB = 2 * R + 1
NCHUNK = R // NQ
MM_DT = os.environ.get("CONV_LSTM_MM_DT", "bf16")  # bf16 | fp32 | fp32r

_CACHE = {}


def _storage_np_dtype():
    import ml_dtypes
    return ml_dtypes.bfloat16 if MM_DT == "bf16" else np.float32


def _prep_core_inputs(x, W, U, b, gamma, beta, moving_mean, moving_var,
                      bidx, half):
    sdt = _storage_np_dtype()
    flip = (half == 1)

    # x slab [T, CIN, SLAB, XW]; XLA SAME (stride2,k3,even) pads bottom/right
    # only: out row r reads input rows 2r..2r+2 (row/col 128 = zero pad).
    xs = np.zeros((T, CIN, SLAB, XW), np.float32)
    xc = np.ascontiguousarray(x[bidx].transpose(0, 3, 1, 2))  # (T,CIN,128,128)
    if not flip:
        xs[:, :, 0:SLAB, 0:128] = xc[:, :, 0:SLAB, :]
    else:
        # slab[s] = x_global[128 - s]; s=0 is the zero pad row
        xs[:, :, 1:SLAB, 0:128] = xc[:, :, 128 - SLAB + 1:][:, :, ::-1, :]

    Wk = W[::-1].copy() if flip else W
    Uk = U[::-1].copy() if flip else U

    # Gate channel packing: m=0 tile holds {o (0:64), i (64:128)},
    # m=1 tile holds {g (0:64), f (64:128)} so i,o share one ACT op and
    # every 2-input DVE op has base-aligned operands (c lives at 64:128).
    sel = [np.r_[192:256, 0:64], np.r_[128:192, 64:128]]

    w3 = np.zeros((96, 768), np.float32)
    ua = np.zeros((128, 768), np.float32)
    ub = np.zeros((128, 768), np.float32)
    # u2 rows 0:64 = U[1]-U[2]: the boundary row reads the AllReduce SUM
    # (own+peer) through U[2] at q=R+1, so its own-h31 tap needs U[1]-U[2]
    # (U2@(own+peer) - U2@own = U2@peer). No subtract, no dup copies.
    u2 = np.zeros((128, 768), np.float32)
    for di in range(3):
        for m in range(2):
            g = di * 2 + m
            cols = slice(g * 128, (g + 1) * 128)
            for j in range(3):
                w3[32*j:32*j+32, cols] = Wk[j, di][:, sel[m]]
            ua[0:64, cols] = Uk[0, di][:, sel[m]]
            ua[64:128, cols] = Uk[1, di][:, sel[m]]
            ub[0:64, cols] = Uk[2, di][:, sel[m]]
            u2[0:64, cols] = (Uk[1, di] - Uk[2, di])[:, sel[m]]

    eps = 1e-3
    scale = (gamma / np.sqrt(moving_var + eps)).astype(np.float32)
    beta2 = (beta - moving_mean * scale).astype(np.float32)
    vecs = np.zeros((128, 8), np.float32)
    # col0: oi ACT bias (hard-sigmoid affine for o rows 0:64, i rows 64:128)
    vecs[0:64, 0] = 0.2 * b[192:256] + 0.5
    vecs[64:128, 0] = 0.2 * b[0:64] + 0.5
    # col1: f ACT bias (input partitions 64:128 of ps1)
    vecs[64:128, 1] = 0.2 * b[64:128] + 0.5
    # col2: g ACT bias (input partitions 0:64 of ps1)
    vecs[0:64, 2] = b[128:192]
    # col3/col4: BN scale / beta for the y affine
    vecs[0:64, 3] = scale
    vecs[0:64, 4] = beta2
    return {
        "xs": np.ascontiguousarray(xs.astype(sdt)),
        "w3": np.ascontiguousarray(w3.astype(sdt)),
        "ua": np.ascontiguousarray(ua.astype(sdt)),
        "ub": np.ascontiguousarray(ub.astype(sdt)),
        "u2": np.ascontiguousarray(u2.astype(sdt)),
        "vecs": vecs,
    }


def _patch_tile_drain():
    """This walrus build encodes at most ONE sync wait per CTRL instruction;
    split the Tile exit drain's waits across SP nops."""
    import bass_rust
    import concourse.tile as tile
    from concourse.vector_clock import ScopedClock
    if getattr(tile.TileContext, "_drain_patched", False):
        return

    def patched(self, tick_clock, wait_clock):
        drain_inst = self.nc.sync.drain()
        wait_clock.add_sem_waits(
            drain_inst.ins, ScopedClock({None: tick_clock.global_clock}))
        si = drain_inst.ins.sync_info
        waits = list(si.on_wait) if si is not None else []
        if len(waits) > 1:
            si.on_wait = waits[:1]
            for w in waits[1:]:
                nop = self.nc.sync.nop()
                nsi = nop.ins.sync_info
                if nsi is None:
                    nop.ins.sync_info = bass_rust.SyncInfo(
                        on_wait=[w], on_update=[])
                else:
                    nsi.on_wait = [w]
        self.nc.all_engine_barrier()
        assert self.sems is not None
        popped = self.nc._tile_sem_poison_stack.pop()
        assert popped is self._sem_poison
        self.nc.clear_and_free_semaphores(list(self.sems.allocated().values()))
        self.nc.all_engine_barrier()

    tile.TileContext._drain_and_barrier = patched
    tile.TileContext._drain_patched = True


def _split_multi_waits(nc, mybir):
    """This walrus build encodes at most one sync wait per instruction;
    move excess waits onto single-wait nops inserted just before."""
    ctr = 0
    for bb in nc.main_func.blocks:
        insts = bb.instructions
        out = []
        changed = False
        for inst in insts:
            si = inst.sync_info
            waits = list(si.on_wait) if si is not None else []
            if len(waits) > 1:
                changed = True
                for w in waits[:-1]:
                    ctr += 1
                    out.append(mybir.InstNoOp(
                        name=f"wsplit-{ctr}",
                        engine=inst.engine,
                        sync_info=mybir.SyncInfo(on_wait=[w], on_update=[]),
                        bass_nofuse=True))
                si.on_wait = [waits[-1]]
            out.append(inst)
        if changed:
            bb.instructions = out


def _build_nc():
    import concourse.bass as bass
    import concourse.mybir as mybir
    import concourse.tile as tile
    _patch_tile_drain()
    dt = mybir.dt
    sdt = dt.bfloat16 if MM_DT == "bf16" else dt.float32
    AF = mybir.ActivationFunctionType

    def mm_ap(ap):
        return ap.bitcast(dt.float32r) if MM_DT == "fp32r" else ap

    nc = bass.Bass()
    xs = nc.dram_tensor("xs", [T, CIN, SLAB, XW], sdt, kind="ExternalInput")
    w3 = nc.dram_tensor("w3", [96, 768], sdt, kind="ExternalInput")
    ua = nc.dram_tensor("ua", [128, 768], sdt, kind="ExternalInput")
    ub = nc.dram_tensor("ub", [128, 768], sdt, kind="ExternalInput")
    u2 = nc.dram_tensor("u2", [128, 768], sdt, kind="ExternalInput")
    vecs = nc.dram_tensor("vecs", [128, 8], dt.float32, kind="ExternalInput")
    y = nc.dram_tensor("y", [T, F, R * W2], dt.float32, kind="ExternalOutput")

    groups = [[0, 1], [2, 3], [4, 5], [6, 7]]

    with tile.TileContext(nc) as tc:
        with (
            tc.tile_pool(name="const", bufs=1) as cpool,
            tc.tile_pool(name="state", bufs=1) as spool,
            tc.tile_pool(name="xp", bufs=2) as xpool,
            tc.tile_pool(name="ps", bufs=7, space="PSUM") as pspool,
            tc.tile_pool(name="epi", bufs=3) as epool,
            tc.tile_pool(name="yp", bufs=4) as ypool,
            tc.tile_pool(name="halo", bufs=2) as hpool,
            tc.tile_pool(name="dram", bufs=2, space="DRAM") as dpool,
        ):
            w3sb = cpool.tile([96, 768], sdt, tag="w3sb")
            uasb = cpool.tile([128, 768], sdt, tag="uasb")
            ubsb = cpool.tile([128, 768], sdt, tag="ubsb")
            u2sb = cpool.tile([128, 768], sdt, tag="u2sb")
            vsb = cpool.tile([128, 8], dt.float32, tag="vsb")
            nc.sync.dma_start(out=w3sb[:], in_=w3[:])
            nc.sync.dma_start(out=uasb[:], in_=ua[:])
            nc.sync.dma_start(out=ubsb[:], in_=ub[:])
            nc.sync.dma_start(out=u2sb[:], in_=u2[:])
            nc.sync.dma_start(out=vsb[:], in_=vecs[:])

            h2 = [spool.tile([128, (R + 2) * WP], sdt, name=f"h2_{i}",
                             tag=f"h2_{i}")
                  for i in range(2)]
            # cell state lives at partitions 64:128 so t1/t2/add are
            # base-aligned with f/i/g (which come out of PSUM upper halves)
            c_sb = spool.tile([128, R * W2], dt.float32, tag="c")
            nc.vector.memset(h2[0][:], 0.0)
            nc.vector.memset(h2[1][:], 0.0)
            nc.vector.memset(c_sb[64:128, :], 0.0)

            def emit_mms(t, q0, nq, hpr, x3r, pss=None):
                bnd = (nq == 1)
                n = nq * W2
                if pss is None:
                    pss = [pspool.tile([128, n], dt.float32,
                                       name=f"ps_{t}_{q0}_{mi}",
                                       tag="ps")[:]
                           for mi in range(2)]
                if bnd:
                    # 12 K<=96 MMs: w3 (x), ua0 = U0@h30, uc = (U1-U2)@h31
                    # (reads hlo of the previous boundary row directly),
                    # u2a = U2@SUM last (the only halo-dependent MMs)
                    stages = [
                        (w3sb, slice(0, 96),
                         lambda d: x3r[0:96, q0:q0+1, d+1:d+129:2]),
                        (uasb, slice(0, 64),
                         lambda d: hpr[0:64, q0:q0+1, 1+d:65+d]),
                        (u2sb, slice(0, 64),
                         lambda d: hpr[0:64, q0+1:q0+2, 1+d:65+d]),
                        (ubsb, slice(0, 64),
                         lambda d: hpr[0:64, q0+2:q0+3, 1+d:65+d]),
                    ]
                    order = [(m, s) for s in range(4) for m in (0, 1)]
                    last = 11
                else:
                    stages = [
                        (w3sb, slice(0, 96),
                         lambda d: x3r[0:96, q0:q0+nq, d+1:d+129:2]),
                        (uasb, slice(0, 128),
                         lambda d: hpr[0:128, q0:q0+nq, 1+d:65+d]),
                        (ubsb, slice(0, 128),
                         lambda d: hpr[0:128, q0+2:q0+nq+2, 1+d:65+d]),
                    ]
                    order = [(m, s) for m in (0, 1) for s in range(3)]
                    last = 8
                nmm = {m: 0 for m in (0, 1)}
                for m, stage in order:
                    psr = pss[m].rearrange("p (a b) -> p a b", b=W2)
                    wsb, rows, rhs_fn = stages[stage]
                    for di in range(3):
                        d = di - 1
                        gcol = slice((di*2+m)*128, (di*2+m+1)*128)
                        nc.tensor.matmul(
                            psr[:], lhsT=mm_ap(wsb[rows, gcol]),
                            rhs=mm_ap(rhs_fn(d)),
                            start=(nmm[m] == 0), stop=(nmm[m] == last))
                        nmm[m] += 1
                return pss

            def emit_partA(t, q0, nq, pss):
                ps0, ps1 = pss
                n = nq * W2
                sfx = "b" if nq == 1 else ""
                cs = slice(q0 * W2, (q0 + nq) * W2)
                # f,g first (they feed the serial t1/t2/add chain), then
                # oi as one [128,n] hard-sigmoid-affine ACT (o at 0:64, i at
                # 64:128; clip via min in the gate products below)
                f_t = epool.tile([128, n], dt.float32, tag="f" + sfx)
                nc.scalar.activation(f_t[64:128, :], ps1[64:128, :], AF.Relu,
                                     bias=vsb[64:128, 1:2], scale=0.2)
                g_t = epool.tile([128, n], dt.float32, tag="g" + sfx)
                nc.scalar.activation(g_t[64:128, :], ps1[0:64, :], AF.Tanh,
                                     bias=vsb[0:64, 2:3], scale=1.0)
                io_t = epool.tile([128, n], dt.float32, tag="io" + sfx)
                nc.scalar.activation(io_t[:], ps0, AF.Relu,
                                     bias=vsb[:, 0:1], scale=0.2)
                t1 = epool.tile([128, n], dt.float32, tag="t1" + sfx)
                nc.vector.scalar_tensor_tensor(
                    t1[64:128, :], f_t[64:128, :], 1.0, c_sb[64:128, cs],
                    mybir.AluOpType.min, mybir.AluOpType.mult)
                t2 = epool.tile([128, n], dt.float32, tag="t2" + sfx)
                nc.vector.scalar_tensor_tensor(
                    t2[64:128, :], io_t[64:128, :], 1.0, g_t[64:128, :],
                    mybir.AluOpType.min, mybir.AluOpType.mult)
                nc.vector.tensor_add(c_sb[64:128, cs], t1[64:128, :],
                                     t2[64:128, :])
                return io_t

            def emit_partB(t, q0, nq, io_t, hcr):
                n = nq * W2
                sfx = "b" if nq == 1 else ""
                cs = slice(q0 * W2, (q0 + nq) * W2)
                tc_t = epool.tile([64, n], dt.float32, tag="tc" + sfx)
                nc.scalar.activation(tc_t[:], c_sb[64:128, cs], AF.Tanh)
                hlo = hcr[0:64, q0+1:q0+nq+1, 1:65]
                nc.vector.scalar_tensor_tensor(
                    hlo,
                    io_t[0:64, :].rearrange("p (a b) -> p a b", b=W2), 1.0,
                    tc_t[:].rearrange("p (a b) -> p a b", b=W2),
                    mybir.AluOpType.min, mybir.AluOpType.mult)
                if nq > 1:
                    nc.vector.tensor_copy(
                        out=hcr[64:128, q0:q0+nq, 1:65], in_=hlo)
                yst = ypool.tile([64, n], dt.float32, tag="yst" + sfx)
                yeng = nc.vector if nq == 1 else nc.gpsimd
                yeng.tensor_scalar(
                    yst[:].rearrange("p (a b) -> p a b", b=W2), hlo,
                    vsb[0:64, 3:4], vsb[0:64, 4:5],
                    mybir.AluOpType.mult, mybir.AluOpType.add)
                nc.sync.dma_start(out=y[t, :, q0*W2:(q0+nq)*W2], in_=yst[:])

            for t in range(T):
                hc = h2[t % 2]
                hp = h2[(t + 1) % 2]
                hcr = hc[:].rearrange("p (q w) -> p q w", w=WP)
                hpr = hp[:].rearrange("p (q w) -> p q w", w=WP)

                x3t = xpool.tile([96, R * XW], sdt, tag="x3")
                x3r = x3t[:].rearrange("p (q w) -> p q w", w=XW)
                nc.sync.dma_start(out=x3r[0:32], in_=xs[t, :, 0:2*R-1:2, :])
                nc.sync.dma_start(out=x3r[32:64], in_=xs[t, :, 1:2*R:2, :])
                nc.sync.dma_start(out=x3r[64:96], in_=xs[t, :, 2:2*R+1:2, :])

                # boundary single-row chunk FIRST: its halo input (bout of
                # step t-1) launched at the top of step t-1, so a full step
                # of interior work covers the mesh latency; likewise the
                # exchange launched here is consumed a full step downstream.
                # Its PSUM shares the c3a tiles' spare columns (448+64=512).
                sh = [pspool.tile([128, 512], dt.float32,
                                  name=f"sh_{t}_{mi}", tag="ps")
                      for mi in range(2)]
                pss = emit_mms(t, R - 1, 1, hpr, x3r,
                               pss=[s[:][:, 448:512] for s in sh])
                io_t = emit_partA(t, R - 1, 1, pss)
                emit_partB(t, R - 1, 1, io_t, hcr)

                # halo exchange (skip after last step): bf16; the AllReduce
                # SUM lands directly at q=R+1 parts 0:64 (u2's -U2 rows
                # subtract own inside the boundary matmul). DMAs + trigger on
                # the GPSIMD queue so waits never block ACT/DVE/PE.
                if t < T - 1:
                    bin_d = dpool.tile([64, 64], sdt, tag="bin")
                    bout_d = dpool.tile([64, 64], sdt, tag="bout")
                    nc.gpsimd.dma_start(out=bin_d[:],
                                        in_=hcr[0:64, R, 1:65])
                    nc.gpsimd.collective_compute(
                        "AllReduce", mybir.AluOpType.add,
                        replica_groups=groups,
                        ins=[bin_d[:].opt()], outs=[bout_d[:].opt()])
                    nc.gpsimd.dma_start(out=hcr[0:64, R+1, 1:65],
                                        in_=bout_d[:])

                # interior chunks (c3a third so the boundary row's h[30]
                # input lands mid-step, and every cross-step h edge has
                # >= half a step of slack); partB emitted one chunk behind
                # partA so no engine FIFO waits cross-chunk.
                chunks = [(0, NQ), (NQ, NQ), (3 * NQ, NQ - 1), (2 * NQ, NQ)]
                prevB = None
                for q0, nq in chunks:
                    cpss = [s[:][:, 0:448] for s in sh] \
                        if nq == NQ - 1 else None
                    pss = emit_mms(t, q0, nq, hpr, x3r, pss=cpss)
                    io_t = emit_partA(t, q0, nq, pss)
                    if prevB is not None:
                        emit_partB(t, prevB[0], prevB[1], prevB[2], hcr)
                    prevB = (q0, nq, io_t)
                emit_partB(t, prevB[0], prevB[1], prevB[2], hcr)
    _split_multi_waits(nc, mybir)
    return nc


def _install_ntff_hook():
    """The image's antenv lacks axon_hooks; synthesize it and register the
    ctypes NTFF profile hook so trace=True works under axon."""
    import sys
    import types
    try:
        from antenv.axon_hooks import get_axon_ntff_profile_hook  # noqa
        return
    except ImportError:
        pass
    mod = types.ModuleType("antenv.axon_hooks")
    mod._hook = None

    def set_axon_ntff_profile_hook(h):
        mod._hook = h

    def get_axon_ntff_profile_hook():
        return mod._hook

    mod.set_axon_ntff_profile_hook = set_axon_ntff_profile_hook
    mod.get_axon_ntff_profile_hook = get_axon_ntff_profile_hook
    sys.modules["antenv.axon_hooks"] = mod
    import antenv
    antenv.axon_hooks = mod
    try:
        from trn_agent_boot.trn_boot import _ntff_profile_via_ctypes
        hook = _ntff_profile_via_ctypes("/opt/axon/libaxon_pjrt.so")
        if hook is not None:
            mod._hook = hook
    except Exception:
        pass


def _get_nc():
    key = (MM_DT,)
    if key not in _CACHE:
        _CACHE[key] = _build_nc()
    return _CACHE[key]


def kernel(x, W, U, b, gamma, beta, moving_mean, moving_var):
    from concourse.bass_utils import run_bass_kernel_spmd
    x = np.asarray(x, np.float32)
    W = np.asarray(W, np.float32)
    U = np.asarray(U, np.float32)
    b = np.asarray(b, np.float32)
    gamma = np.asarray(gamma, np.float32)
    beta = np.asarray(beta, np.float32)
    moving_mean = np.asarray(moving_mean, np.float32)
    moving_var = np.asarray(moving_var, np.float32)
    B = x.shape[0]

    in_maps = []
    for bidx in range(B):
        for half in range(N_HALVES):
            in_maps.append(_prep_core_inputs(
                x, W, U, b, gamma, beta, moving_mean, moving_var, bidx, half))

    nc = _get_nc()
    trace = os.environ.get("BASS_KERNEL_TRACE") == "1"
    if trace:
        _install_ntff_hook()
    res = run_bass_kernel_spmd(nc, in_maps, core_ids=list(range(8)),
                               trace=trace)
    kernel._last_result = res

    out = np.zeros((B, T, H2, W2, F), np.float32)
    ci = 0
    for bidx in range(B):
        for half in range(N_HALVES):
            yc = res.results[ci]["y"].reshape(T, F, R, W2)
            ci += 1
            yc = yc.transpose(0, 2, 3, 1)  # (T, R, W2, F)
            if half == 1:
                yc = yc[:, ::-1, :, :]
                out[bidx, :, 32:64] = yc
            else:
                out[bidx, :, 0:32] = yc
    return out


# revision 32
# speedup vs baseline: 1.0515x; 1.0046x over previous
"""ConvLSTM block Trainium2 kernel (8 NeuronCores).

Sharding: 8 cores = 4 batches x 2 H-halves. Bottom-half cores process their
slab vertically flipped (with kh-flipped conv kernels) so one SPMD program
serves all cores. Per timestep the two halves of a batch exchange one
boundary row of h via a pairwise AllReduce (halo = sum - own, parity-free).

Per-core compute per step: for each of 2 output-channel tiles (128 ch) and
each chunk of 8 output rows (N=512 pixels), one PSUM accumulation group of
9 matmuls: 3x input conv (K=96: 3 row-taps x 32ch; stride-2 column access
via strided APs) + 6x recurrent conv (K=128: 2 row-taps x 64ch using a
row-shifted duplicate copy of h in partitions 64:128).

Schedule: the halo-consuming boundary chunk runs LAST each step so the
AllReduce latency is covered by the interior chunks; the whole halo path
(DMAs + collective + sub in bf16) lives on the GPSIMD queue so its waits
never block the ACT/DVE FIFOs. Gates are packed {i,o},{f,g} so i and o
activate in one [128,512] ACT op; BN affine and the h dup-copy run on
GPSIMD via tensor_scalar/tensor_copy. Epilogues are emitted split
(partA/partB) one chunk apart so no engine FIFO waits cross-chunk.
"""
import os
import numpy as np

T, H2, W2, F, CIN = 16, 64, 64, 64, 32
WP, XW, NQ = 66, 130, 8
N_HALVES = 2
R = H2 // N_HALVES
SLAB = 2 * R + 1
NCHUNK = R // NQ
MM_DT = os.environ.get("CONV_LSTM_MM_DT", "bf16")  # bf16 | fp32 | fp32r

_CACHE = {}


def _storage_np_dtype():
    import ml_dtypes
    return ml_dtypes.bfloat16 if MM_DT == "bf16" else np.float32


def _prep_core_inputs(x, W, U, b, gamma, beta, moving_mean, moving_var,
                      bidx, half):
    sdt = _storage_np_dtype()
    flip = (half == 1)

    # x slab [T, CIN, SLAB, XW]; XLA SAME (stride2,k3,even) pads bottom/right
    # only: out row r reads input rows 2r..2r+2 (row/col 128 = zero pad).
    xs = np.zeros((T, CIN, SLAB, XW), np.float32)
    xc = np.ascontiguousarray(x[bidx].transpose(0, 3, 1, 2))  # (T,CIN,128,128)
    if not flip:
        xs[:, :, 0:SLAB, 0:128] = xc[:, :, 0:SLAB, :]
    else:
        # slab[s] = x_global[128 - s]; s=0 is the zero pad row
        xs[:, :, 1:SLAB, 0:128] = xc[:, :, 128 - SLAB + 1:][:, :, ::-1, :]

    Wk = W[::-1].copy() if flip else W
    Uk = U[::-1].copy() if flip else U

    # Gate channel packing: m=0 tile holds {o (0:64), i (64:128)},
    # m=1 tile holds {g (0:64), f (64:128)} so i,o share one ACT op and
    # every 2-input DVE op has base-aligned operands (c lives at 64:128).
    sel = [np.r_[192:256, 0:64], np.r_[128:192, 64:128]]

    w3 = np.zeros((96, 768), np.float32)
    ua = np.zeros((128, 768), np.float32)
    ub = np.zeros((128, 768), np.float32)
    # u2 rows 0:64 = U[1]-U[2]: the boundary row reads the AllReduce SUM
    # (own+peer) through U[2] at q=R+1, so its own-h31 tap needs U[1]-U[2]
    # (U2@(own+peer) - U2@own = U2@peer). No subtract, no dup copies.
    u2 = np.zeros((128, 768), np.float32)
    for di in range(3):
        for m in range(2):
            g = di * 2 + m
            cols = slice(g * 128, (g + 1) * 128)
            for j in range(3):
                w3[32*j:32*j+32, cols] = Wk[j, di][:, sel[m]]
            ua[0:64, cols] = Uk[0, di][:, sel[m]]
            ua[64:128, cols] = Uk[1, di][:, sel[m]]
            ub[0:64, cols] = Uk[2, di][:, sel[m]]
            u2[0:64, cols] = (Uk[1, di] - Uk[2, di])[:, sel[m]]

    eps = 1e-3
    scale = (gamma / np.sqrt(moving_var + eps)).astype(np.float32)
    beta2 = (beta - moving_mean * scale).astype(np.float32)
    vecs = np.zeros((128, 8), np.float32)
    # col0: oi ACT bias (hard-sigmoid affine for o rows 0:64, i rows 64:128)
    vecs[0:64, 0] = 0.2 * b[192:256] + 0.5
    vecs[64:128, 0] = 0.2 * b[0:64] + 0.5
    # col1: f ACT bias (input partitions 64:128 of ps1)
    vecs[64:128, 1] = 0.2 * b[64:128] + 0.5
    # col2: g ACT bias (input partitions 0:64 of ps1)
    vecs[0:64, 2] = b[128:192]
    # col3/col4: BN scale / beta for the y affine
    vecs[0:64, 3] = scale
    vecs[0:64, 4] = beta2
    return {
        "xs": np.ascontiguousarray(xs.astype(sdt)),
        "w3": np.ascontiguousarray(w3.astype(sdt)),
        "ua": np.ascontiguousarray(ua.astype(sdt)),
        "ub": np.ascontiguousarray(ub.astype(sdt)),
        "u2": np.ascontiguousarray(u2.astype(sdt)),
        "vecs": vecs,
    }


def _patch_tile_drain():
    """This walrus build encodes at most ONE sync wait per CTRL instruction;
    split the Tile exit drain's waits across SP nops."""
    import bass_rust
    import concourse.tile as tile
    from concourse.vector_clock import ScopedClock
    if getattr(tile.TileContext, "_drain_patched", False):
        return

    def patched(self, tick_clock, wait_clock):
        drain_inst = self.nc.sync.drain()
        wait_clock.add_sem_waits(
            drain_inst.ins, ScopedClock({None: tick_clock.global_clock}))
        si = drain_inst.ins.sync_info
        waits = list(si.on_wait) if si is not None else []
        if len(waits) > 1:
            si.on_wait = waits[:1]
            for w in waits[1:]:
                nop = self.nc.sync.nop()
                nsi = nop.ins.sync_info
                if nsi is None:
                    nop.ins.sync_info = bass_rust.SyncInfo(
                        on_wait=[w], on_update=[])
                else:
                    nsi.on_wait = [w]
        self.nc.all_engine_barrier()
        assert self.sems is not None
        popped = self.nc._tile_sem_poison_stack.pop()
        assert popped is self._sem_poison
        self.nc.clear_and_free_semaphores(list(self.sems.allocated().values()))
        self.nc.all_engine_barrier()

    tile.TileContext._drain_and_barrier = patched
    tile.TileContext._drain_patched = True


def _split_multi_waits(nc, mybir):
    """This walrus build encodes at most one sync wait per instruction;
    move excess waits onto single-wait nops inserted just before."""
    ctr = 0
    for bb in nc.main_func.blocks:
        insts = bb.instructions
        out = []
        changed = False
        for inst in insts:
            si = inst.sync_info
            waits = list(si.on_wait) if si is not None else []
            if len(waits) > 1:
                changed = True
                for w in waits[:-1]:
                    ctr += 1
                    out.append(mybir.InstNoOp(
                        name=f"wsplit-{ctr}",
                        engine=inst.engine,
                        sync_info=mybir.SyncInfo(on_wait=[w], on_update=[]),
                        bass_nofuse=True))
                si.on_wait = [waits[-1]]
            out.append(inst)
        if changed:
            bb.instructions = out


def _build_nc():
    import concourse.bass as bass
    import concourse.mybir as mybir
    import concourse.tile as tile
    _patch_tile_drain()
    dt = mybir.dt
    sdt = dt.bfloat16 if MM_DT == "bf16" else dt.float32
    AF = mybir.ActivationFunctionType

    def mm_ap(ap):
        return ap.bitcast(dt.float32r) if MM_DT == "fp32r" else ap

    nc = bass.Bass()
    xs = nc.dram_tensor("xs", [T, CIN, SLAB, XW], sdt, kind="ExternalInput")
    w3 = nc.dram_tensor("w3", [96, 768], sdt, kind="ExternalInput")
    ua = nc.dram_tensor("ua", [128, 768], sdt, kind="ExternalInput")
    ub = nc.dram_tensor("ub", [128, 768], sdt, kind="ExternalInput")
    u2 = nc.dram_tensor("u2", [128, 768], sdt, kind="ExternalInput")
    vecs = nc.dram_tensor("vecs", [128, 8], dt.float32, kind="ExternalInput")
    y = nc.dram_tensor("y", [T, F, R * W2], dt.float32, kind="ExternalOutput")

    groups = [[0, 1], [2, 3], [4, 5], [6, 7]]

    with tile.TileContext(nc) as tc:
        with (
            tc.tile_pool(name="const", bufs=1) as cpool,
            tc.tile_pool(name="state", bufs=1) as spool,
            tc.tile_pool(name="xp", bufs=2) as xpool,
            tc.tile_pool(name="ps", bufs=8, space="PSUM") as pspool,
            tc.tile_pool(name="epi", bufs=3) as epool,
            tc.tile_pool(name="yp", bufs=4) as ypool,
            tc.tile_pool(name="halo", bufs=2) as hpool,
            tc.tile_pool(name="dram", bufs=2, space="DRAM") as dpool,
        ):
            w3sb = cpool.tile([96, 768], sdt, tag="w3sb")
            uasb = cpool.tile([128, 768], sdt, tag="uasb")
            ubsb = cpool.tile([128, 768], sdt, tag="ubsb")
            u2sb = cpool.tile([128, 768], sdt, tag="u2sb")
            vsb = cpool.tile([128, 8], dt.float32, tag="vsb")
            nc.sync.dma_start(out=w3sb[:], in_=w3[:])
            nc.sync.dma_start(out=uasb[:], in_=ua[:])
            nc.sync.dma_start(out=ubsb[:], in_=ub[:])
            nc.sync.dma_start(out=u2sb[:], in_=u2[:])
            nc.sync.dma_start(out=vsb[:], in_=vecs[:])

            h2 = [spool.tile([128, (R + 2) * WP], sdt, name=f"h2_{i}",
                             tag=f"h2_{i}")
                  for i in range(2)]
            # cell state lives at partitions 64:128 so t1/t2/add are
            # base-aligned with f/i/g (which come out of PSUM upper halves)
            c_sb = spool.tile([128, R * W2], dt.float32, tag="c")
            nc.vector.memset(h2[0][:], 0.0)
            nc.vector.memset(h2[1][:], 0.0)
            nc.vector.memset(c_sb[64:128, :], 0.0)

            def emit_mms(t, q0, nq, hpr, x3r, pss=None):
                bnd = (nq == 1)
                n = nq * W2
                if pss is None:
                    pss = [pspool.tile([128, n], dt.float32,
                                       name=f"ps_{t}_{q0}_{mi}",
                                       tag="ps")[:]
                           for mi in range(2)]
                if bnd:
                    # 12 K<=96 MMs: w3 (x), ua0 = U0@h30, uc = (U1-U2)@h31
                    # (reads hlo of the previous boundary row directly),
                    # u2a = U2@SUM last (the only halo-dependent MMs)
                    stages = [
                        (w3sb, slice(0, 96),
                         lambda d: x3r[0:96, q0:q0+1, d+1:d+129:2]),
                        (uasb, slice(0, 64),
                         lambda d: hpr[0:64, q0:q0+1, 1+d:65+d]),
                        (u2sb, slice(0, 64),
                         lambda d: hpr[0:64, q0+1:q0+2, 1+d:65+d]),
                        (ubsb, slice(0, 64),
                         lambda d: hpr[0:64, q0+2:q0+3, 1+d:65+d]),
                    ]
                    order = [(m, s) for s in range(4) for m in (0, 1)]
                    last = 11
                else:
                    stages = [
                        (w3sb, slice(0, 96),
                         lambda d: x3r[0:96, q0:q0+nq, d+1:d+129:2]),
                        (uasb, slice(0, 128),
                         lambda d: hpr[0:128, q0:q0+nq, 1+d:65+d]),
                        (ubsb, slice(0, 128),
                         lambda d: hpr[0:128, q0+2:q0+nq+2, 1+d:65+d]),
                    ]
                    order = [(m, s) for m in (0, 1) for s in range(3)]
                    last = 8
                nmm = {m: 0 for m in (0, 1)}
                for m, stage in order:
                    psr = pss[m].rearrange("p (a b) -> p a b", b=W2)
                    wsb, rows, rhs_fn = stages[stage]
                    for di in range(3):
                        d = di - 1
                        gcol = slice((di*2+m)*128, (di*2+m+1)*128)
                        nc.tensor.matmul(
                            psr[:], lhsT=mm_ap(wsb[rows, gcol]),
                            rhs=mm_ap(rhs_fn(d)),
                            start=(nmm[m] == 0), stop=(nmm[m] == last))
                        nmm[m] += 1
                return pss

            def emit_partA(t, q0, nq, pss):
                ps0, ps1 = pss
                n = nq * W2
                sfx = "b" if nq == 1 else ""
                cs = slice(q0 * W2, (q0 + nq) * W2)
                # f,g first (they feed the serial t1/t2/add chain), then
                # oi as one [128,n] hard-sigmoid-affine ACT (o at 0:64, i at
                # 64:128; clip via min in the gate products below)
                f_t = epool.tile([128, n], dt.float32, tag="f" + sfx)
                nc.scalar.activation(f_t[64:128, :], ps1[64:128, :], AF.Relu,
                                     bias=vsb[64:128, 1:2], scale=0.2)
                g_t = epool.tile([128, n], dt.float32, tag="g" + sfx)
                nc.scalar.activation(g_t[64:128, :], ps1[0:64, :], AF.Tanh,
                                     bias=vsb[0:64, 2:3], scale=1.0)
                io_t = epool.tile([128, n], dt.float32, tag="io" + sfx)
                nc.scalar.activation(io_t[:], ps0, AF.Relu,
                                     bias=vsb[:, 0:1], scale=0.2)
                t1 = epool.tile([128, n], dt.float32, tag="t1" + sfx)
                nc.vector.scalar_tensor_tensor(
                    t1[64:128, :], f_t[64:128, :], 1.0, c_sb[64:128, cs],
                    mybir.AluOpType.min, mybir.AluOpType.mult)
                t2 = epool.tile([128, n], dt.float32, tag="t2" + sfx)
                nc.vector.scalar_tensor_tensor(
                    t2[64:128, :], io_t[64:128, :], 1.0, g_t[64:128, :],
                    mybir.AluOpType.min, mybir.AluOpType.mult)
                nc.vector.tensor_add(c_sb[64:128, cs], t1[64:128, :],
                                     t2[64:128, :])
                return io_t

            def emit_partB(t, q0, nq, io_t, hcr):
                n = nq * W2
                sfx = "b" if nq == 1 else ""
                cs = slice(q0 * W2, (q0 + nq) * W2)
                tc_t = epool.tile([64, n], dt.float32, tag="tc" + sfx)
                nc.scalar.activation(tc_t[:], c_sb[64:128, cs], AF.Tanh)
                hlo = hcr[0:64, q0+1:q0+nq+1, 1:65]
                nc.vector.scalar_tensor_tensor(
                    hlo,
                    io_t[0:64, :].rearrange("p (a b) -> p a b", b=W2), 1.0,
                    tc_t[:].rearrange("p (a b) -> p a b", b=W2),
                    mybir.AluOpType.min, mybir.AluOpType.mult)
                if nq > 1:
                    nc.vector.tensor_copy(
                        out=hcr[64:128, q0:q0+nq, 1:65], in_=hlo)
                yst = ypool.tile([64, n], dt.float32, tag="yst" + sfx)
                yeng = nc.vector if nq == 1 else nc.gpsimd
                yeng.tensor_scalar(
                    yst[:].rearrange("p (a b) -> p a b", b=W2), hlo,
                    vsb[0:64, 3:4], vsb[0:64, 4:5],
                    mybir.AluOpType.mult, mybir.AluOpType.add)
                nc.sync.dma_start(out=y[t, :, q0*W2:(q0+nq)*W2], in_=yst[:])

            for t in range(T):
                hc = h2[t % 2]
                hp = h2[(t + 1) % 2]
                hcr = hc[:].rearrange("p (q w) -> p q w", w=WP)
                hpr = hp[:].rearrange("p (q w) -> p q w", w=WP)

                x3t = xpool.tile([96, R * XW], sdt, tag="x3")
                x3r = x3t[:].rearrange("p (q w) -> p q w", w=XW)
                nc.sync.dma_start(out=x3r[0:32], in_=xs[t, :, 0:2*R-1:2, :])
                nc.sync.dma_start(out=x3r[32:64], in_=xs[t, :, 1:2*R:2, :])
                nc.sync.dma_start(out=x3r[64:96], in_=xs[t, :, 2:2*R+1:2, :])

                # boundary single-row chunk FIRST: its halo input (bout of
                # step t-1) launched at the top of step t-1, so a full step
                # of interior work covers the mesh latency; likewise the
                # exchange launched here is consumed a full step downstream.
                # Its PSUM shares the c3a tiles' spare columns (448+64=512).
                sh = [pspool.tile([128, 512], dt.float32,
                                  name=f"sh_{t}_{mi}", tag="ps")
                      for mi in range(2)]
                pss = emit_mms(t, R - 1, 1, hpr, x3r,
                               pss=[s[:][:, 448:512] for s in sh])
                io_t = emit_partA(t, R - 1, 1, pss)
                emit_partB(t, R - 1, 1, io_t, hcr)

                # halo exchange (skip after last step): bf16; the AllReduce
                # SUM lands directly at q=R+1 parts 0:64 (u2's -U2 rows
                # subtract own inside the boundary matmul). DMAs + trigger on
                # the GPSIMD queue so waits never block ACT/DVE/PE.
                if t < T - 1:
                    bin_d = dpool.tile([64, 64], sdt, tag="bin")
                    bout_d = dpool.tile([64, 64], sdt, tag="bout")
                    nc.gpsimd.dma_start(out=bin_d[:],
                                        in_=hcr[0:64, R, 1:65])
                    nc.gpsimd.collective_compute(
                        "AllReduce", mybir.AluOpType.add,
                        replica_groups=groups,
                        ins=[bin_d[:].opt()], outs=[bout_d[:].opt()])
                    nc.gpsimd.dma_start(out=hcr[0:64, R+1, 1:65],
                                        in_=bout_d[:])

                # interior chunks (c3a third so the boundary row's h[30]
                # input lands mid-step, and every cross-step h edge has
                # >= half a step of slack); partB emitted one chunk behind
                # partA so no engine FIFO waits cross-chunk.
                chunks = [(0, NQ), (NQ, NQ), (3 * NQ, NQ - 1), (2 * NQ, NQ)]
                prevB = None
                for q0, nq in chunks:
                    cpss = [s[:][:, 0:448] for s in sh] \
                        if nq == NQ - 1 else None
                    pss = emit_mms(t, q0, nq, hpr, x3r, pss=cpss)
                    io_t = emit_partA(t, q0, nq, pss)
                    if prevB is not None:
                        emit_partB(t, prevB[0], prevB[1], prevB[2], hcr)
                    prevB = (q0, nq, io_t)
                emit_partB(t, prevB[0], prevB[1], prevB[2], hcr)
    _split_multi_waits(nc, mybir)
    return nc


def _install_ntff_hook():
    """The image's antenv lacks axon_hooks; synthesize it and register the
    ctypes NTFF profile hook so trace=True works under axon."""
    import sys
    import types
    try:
        from antenv.axon_hooks import get_axon_ntff_profile_hook  # noqa
        return
    except ImportError:
        pass
    mod = types.ModuleType("antenv.axon_hooks")
    mod._hook = None

    def set_axon_ntff_profile_hook(h):
        mod._hook = h

    def get_axon_ntff_profile_hook():
        return mod._hook

    mod.set_axon_ntff_profile_hook = set_axon_ntff_profile_hook
    mod.get_axon_ntff_profile_hook = get_axon_ntff_profile_hook
    sys.modules["antenv.axon_hooks"] = mod
    import antenv
    antenv.axon_hooks = mod
    try:
        from trn_agent_boot.trn_boot import _ntff_profile_via_ctypes
        hook = _ntff_profile_via_ctypes("/opt/axon/libaxon_pjrt.so")
        if hook is not None:
            mod._hook = hook
    except Exception:
        pass


def _get_nc():
    key = (MM_DT,)
    if key not in _CACHE:
        _CACHE[key] = _build_nc()
    return _CACHE[key]


def kernel(x, W, U, b, gamma, beta, moving_mean, moving_var):
    from concourse.bass_utils import run_bass_kernel_spmd
    x = np.asarray(x, np.float32)
    W = np.asarray(W, np.float32)
    U = np.asarray(U, np.float32)
    b = np.asarray(b, np.float32)
    gamma = np.asarray(gamma, np.float32)
    beta = np.asarray(beta, np.float32)
    moving_mean = np.asarray(moving_mean, np.float32)
    moving_var = np.asarray(moving_var, np.float32)
    B = x.shape[0]

    in_maps = []
    for bidx in range(B):
        for half in range(N_HALVES):
            in_maps.append(_prep_core_inputs(
                x, W, U, b, gamma, beta, moving_mean, moving_var, bidx, half))

    nc = _get_nc()
    trace = os.environ.get("BASS_KERNEL_TRACE") == "1"
    if trace:
        _install_ntff_hook()
    res = run_bass_kernel_spmd(nc, in_maps, core_ids=list(range(8)),
                               trace=trace)
    kernel._last_result = res

    out = np.zeros((B, T, H2, W2, F), np.float32)
    ci = 0
    for bidx in range(B):
        for half in range(N_HALVES):
            yc = res.results[ci]["y"].reshape(T, F, R, W2)
            ci += 1
            yc = yc.transpose(0, 2, 3, 1)  # (T, R, W2, F)
            if half == 1:
                yc = yc[:, ::-1, :, :]
                out[bidx, :, 32:64] = yc
            else:
                out[bidx, :, 0:32] = yc
    return out


# revision 33
# speedup vs baseline: 1.0601x; 1.0082x over previous
"""ConvLSTM block Trainium2 kernel (8 NeuronCores).

Sharding: 8 cores = 4 batches x 2 H-halves. Bottom-half cores process their
slab vertically flipped (with kh-flipped conv kernels) so one SPMD program
serves all cores. Per timestep the two halves of a batch exchange one
boundary row of h via a pairwise AllReduce (halo = sum - own, parity-free).

Per-core compute per step: for each of 2 output-channel tiles (128 ch) and
each chunk of 8 output rows (N=512 pixels), one PSUM accumulation group of
9 matmuls: 3x input conv (K=96: 3 row-taps x 32ch; stride-2 column access
via strided APs) + 6x recurrent conv (K=128: 2 row-taps x 64ch using a
row-shifted duplicate copy of h in partitions 64:128).

Schedule: the halo-consuming boundary chunk runs LAST each step so the
AllReduce latency is covered by the interior chunks; the whole halo path
(DMAs + collective + sub in bf16) lives on the GPSIMD queue so its waits
never block the ACT/DVE FIFOs. Gates are packed {i,o},{f,g} so i and o
activate in one [128,512] ACT op; BN affine and the h dup-copy run on
GPSIMD via tensor_scalar/tensor_copy. Epilogues are emitted split
(partA/partB) one chunk apart so no engine FIFO waits cross-chunk.
"""
import os
import numpy as np

T, H2, W2, F, CIN = 16, 64, 64, 64, 32
WP, XW, NQ = 66, 130, 8
N_HALVES = 2
R = H2 // N_HALVES
SLAB = 2 * R + 1
NCHUNK = R // NQ
MM_DT = os.environ.get("CONV_LSTM_MM_DT", "bf16")  # bf16 | fp32 | fp32r

_CACHE = {}


def _storage_np_dtype():
    import ml_dtypes
    return ml_dtypes.bfloat16 if MM_DT == "bf16" else np.float32


def _prep_core_inputs(x, W, U, b, gamma, beta, moving_mean, moving_var,
                      bidx, half):
    sdt = _storage_np_dtype()
    flip = (half == 1)

    # x slab [T, CIN, SLAB, XW]; XLA SAME (stride2,k3,even) pads bottom/right
    # only: out row r reads input rows 2r..2r+2 (row/col 128 = zero pad).
    xs = np.zeros((T, CIN, SLAB, XW), np.float32)
    xc = np.ascontiguousarray(x[bidx].transpose(0, 3, 1, 2))  # (T,CIN,128,128)
    if not flip:
        xs[:, :, 0:SLAB, 0:128] = xc[:, :, 0:SLAB, :]
    else:
        # slab[s] = x_global[128 - s]; s=0 is the zero pad row
        xs[:, :, 1:SLAB, 0:128] = xc[:, :, 128 - SLAB + 1:][:, :, ::-1, :]

    Wk = W[::-1].copy() if flip else W
    Uk = U[::-1].copy() if flip else U

    # Gate channel packing: m=0 tile holds {o (0:64), i (64:128)},
    # m=1 tile holds {g (0:64), f (64:128)} so i,o share one ACT op and
    # every 2-input DVE op has base-aligned operands (c lives at 64:128).
    sel = [np.r_[192:256, 0:64], np.r_[128:192, 64:128]]

    w3 = np.zeros((96, 768), np.float32)
    ua = np.zeros((128, 768), np.float32)
    ub = np.zeros((128, 768), np.float32)
    # u2 rows 0:64 = U[1]-U[2]: the boundary row reads the AllReduce SUM
    # (own+peer) through U[2] at q=R+1, so its own-h31 tap needs U[1]-U[2]
    # (U2@(own+peer) - U2@own = U2@peer). No subtract, no dup copies.
    u2 = np.zeros((128, 768), np.float32)
    for di in range(3):
        for m in range(2):
            g = di * 2 + m
            cols = slice(g * 128, (g + 1) * 128)
            for j in range(3):
                w3[32*j:32*j+32, cols] = Wk[j, di][:, sel[m]]
            ua[0:64, cols] = Uk[0, di][:, sel[m]]
            ua[64:128, cols] = Uk[1, di][:, sel[m]]
            ub[0:64, cols] = Uk[2, di][:, sel[m]]
            u2[0:64, cols] = (Uk[1, di] - Uk[2, di])[:, sel[m]]

    eps = 1e-3
    scale = (gamma / np.sqrt(moving_var + eps)).astype(np.float32)
    beta2 = (beta - moving_mean * scale).astype(np.float32)
    vecs = np.zeros((128, 8), np.float32)
    # col0: oi ACT bias (hard-sigmoid affine for o rows 0:64, i rows 64:128)
    vecs[0:64, 0] = 0.2 * b[192:256] + 0.5
    vecs[64:128, 0] = 0.2 * b[0:64] + 0.5
    # col1: f ACT bias (input partitions 64:128 of ps1)
    vecs[64:128, 1] = 0.2 * b[64:128] + 0.5
    # col2: g ACT bias (input partitions 0:64 of ps1)
    vecs[0:64, 2] = b[128:192]
    # col3/col4: BN scale / beta for the y affine
    vecs[0:64, 3] = scale
    vecs[0:64, 4] = beta2
    return {
        "xs": np.ascontiguousarray(xs.astype(sdt)),
        "w3": np.ascontiguousarray(w3.astype(sdt)),
        "ua": np.ascontiguousarray(ua.astype(sdt)),
        "ub": np.ascontiguousarray(ub.astype(sdt)),
        "u2": np.ascontiguousarray(u2.astype(sdt)),
        "vecs": vecs,
    }


def _patch_tile_drain():
    """This walrus build encodes at most ONE sync wait per CTRL instruction;
    split the Tile exit drain's waits across SP nops."""
    import bass_rust
    import concourse.tile as tile
    from concourse.vector_clock import ScopedClock
    if getattr(tile.TileContext, "_drain_patched", False):
        return

    def patched(self, tick_clock, wait_clock):
        drain_inst = self.nc.sync.drain()
        wait_clock.add_sem_waits(
            drain_inst.ins, ScopedClock({None: tick_clock.global_clock}))
        si = drain_inst.ins.sync_info
        waits = list(si.on_wait) if si is not None else []
        if len(waits) > 1:
            si.on_wait = waits[:1]
            for w in waits[1:]:
                nop = self.nc.sync.nop()
                nsi = nop.ins.sync_info
                if nsi is None:
                    nop.ins.sync_info = bass_rust.SyncInfo(
                        on_wait=[w], on_update=[])
                else:
                    nsi.on_wait = [w]
        self.nc.all_engine_barrier()
        assert self.sems is not None
        popped = self.nc._tile_sem_poison_stack.pop()
        assert popped is self._sem_poison
        self.nc.clear_and_free_semaphores(list(self.sems.allocated().values()))
        self.nc.all_engine_barrier()

    tile.TileContext._drain_and_barrier = patched
    tile.TileContext._drain_patched = True


def _split_multi_waits(nc, mybir):
    """This walrus build encodes at most one sync wait per instruction;
    move excess waits onto single-wait nops inserted just before."""
    ctr = 0
    for bb in nc.main_func.blocks:
        insts = bb.instructions
        out = []
        changed = False
        for inst in insts:
            si = inst.sync_info
            waits = list(si.on_wait) if si is not None else []
            if len(waits) > 1:
                changed = True
                for w in waits[:-1]:
                    ctr += 1
                    out.append(mybir.InstNoOp(
                        name=f"wsplit-{ctr}",
                        engine=inst.engine,
                        sync_info=mybir.SyncInfo(on_wait=[w], on_update=[]),
                        bass_nofuse=True))
                si.on_wait = [waits[-1]]
            out.append(inst)
        if changed:
            bb.instructions = out


def _build_nc():
    import concourse.bass as bass
    import concourse.mybir as mybir
    import concourse.tile as tile
    _patch_tile_drain()
    dt = mybir.dt
    sdt = dt.bfloat16 if MM_DT == "bf16" else dt.float32
    AF = mybir.ActivationFunctionType

    def mm_ap(ap):
        return ap.bitcast(dt.float32r) if MM_DT == "fp32r" else ap

    nc = bass.Bass()
    xs = nc.dram_tensor("xs", [T, CIN, SLAB, XW], sdt, kind="ExternalInput")
    w3 = nc.dram_tensor("w3", [96, 768], sdt, kind="ExternalInput")
    ua = nc.dram_tensor("ua", [128, 768], sdt, kind="ExternalInput")
    ub = nc.dram_tensor("ub", [128, 768], sdt, kind="ExternalInput")
    u2 = nc.dram_tensor("u2", [128, 768], sdt, kind="ExternalInput")
    vecs = nc.dram_tensor("vecs", [128, 8], dt.float32, kind="ExternalInput")
    y = nc.dram_tensor("y", [T, F, R * W2], dt.float32, kind="ExternalOutput")

    groups = [[0, 1], [2, 3], [4, 5], [6, 7]]

    with tile.TileContext(nc) as tc:
        with (
            tc.tile_pool(name="const", bufs=1) as cpool,
            tc.tile_pool(name="state", bufs=1) as spool,
            tc.tile_pool(name="xp", bufs=2) as xpool,
            tc.tile_pool(name="ps", bufs=8, space="PSUM") as pspool,
            tc.tile_pool(name="epi", bufs=4) as epool,
            tc.tile_pool(name="yp", bufs=6) as ypool,
            tc.tile_pool(name="halo", bufs=2) as hpool,
            tc.tile_pool(name="dram", bufs=2, space="DRAM") as dpool,
        ):
            w3sb = cpool.tile([96, 768], sdt, tag="w3sb")
            uasb = cpool.tile([128, 768], sdt, tag="uasb")
            ubsb = cpool.tile([128, 768], sdt, tag="ubsb")
            u2sb = cpool.tile([128, 768], sdt, tag="u2sb")
            vsb = cpool.tile([128, 8], dt.float32, tag="vsb")
            nc.sync.dma_start(out=w3sb[:], in_=w3[:])
            nc.sync.dma_start(out=uasb[:], in_=ua[:])
            nc.sync.dma_start(out=ubsb[:], in_=ub[:])
            nc.sync.dma_start(out=u2sb[:], in_=u2[:])
            nc.sync.dma_start(out=vsb[:], in_=vecs[:])

            h2 = [spool.tile([128, (R + 2) * WP], sdt, name=f"h2_{i}",
                             tag=f"h2_{i}")
                  for i in range(2)]
            # cell state lives at partitions 64:128 so t1/t2/add are
            # base-aligned with f/i/g (which come out of PSUM upper halves)
            c_sb = spool.tile([128, R * W2], dt.float32, tag="c")
            nc.vector.memset(h2[0][:], 0.0)
            nc.vector.memset(h2[1][:], 0.0)
            nc.vector.memset(c_sb[64:128, :], 0.0)

            def emit_mms(t, q0, nq, hpr, x3r, pss=None):
                bnd = (nq == 1)
                n = nq * W2
                if pss is None:
                    pss = [pspool.tile([128, n], dt.float32,
                                       name=f"ps_{t}_{q0}_{mi}",
                                       tag="ps")[:]
                           for mi in range(2)]
                if bnd:
                    # 12 K<=96 MMs: w3 (x), ua0 = U0@h30, uc = (U1-U2)@h31
                    # (reads hlo of the previous boundary row directly),
                    # u2a = U2@SUM last (the only halo-dependent MMs)
                    stages = [
                        (w3sb, slice(0, 96),
                         lambda d: x3r[0:96, q0:q0+1, d+1:d+129:2]),
                        (uasb, slice(0, 64),
                         lambda d: hpr[0:64, q0:q0+1, 1+d:65+d]),
                        (u2sb, slice(0, 64),
                         lambda d: hpr[0:64, q0+1:q0+2, 1+d:65+d]),
                        (ubsb, slice(0, 64),
                         lambda d: hpr[0:64, q0+2:q0+3, 1+d:65+d]),
                    ]
                    order = [(m, s) for s in range(4) for m in (0, 1)]
                    last = 11
                else:
                    stages = [
                        (w3sb, slice(0, 96),
                         lambda d: x3r[0:96, q0:q0+nq, d+1:d+129:2]),
                        (uasb, slice(0, 128),
                         lambda d: hpr[0:128, q0:q0+nq, 1+d:65+d]),
                        (ubsb, slice(0, 128),
                         lambda d: hpr[0:128, q0+2:q0+nq+2, 1+d:65+d]),
                    ]
                    order = [(m, s) for m in (0, 1) for s in range(3)]
                    last = 8
                nmm = {m: 0 for m in (0, 1)}
                for m, stage in order:
                    psr = pss[m].rearrange("p (a b) -> p a b", b=W2)
                    wsb, rows, rhs_fn = stages[stage]
                    for di in range(3):
                        d = di - 1
                        gcol = slice((di*2+m)*128, (di*2+m+1)*128)
                        nc.tensor.matmul(
                            psr[:], lhsT=mm_ap(wsb[rows, gcol]),
                            rhs=mm_ap(rhs_fn(d)),
                            start=(nmm[m] == 0), stop=(nmm[m] == last))
                        nmm[m] += 1
                return pss

            def emit_partA(t, q0, nq, pss):
                ps0, ps1 = pss
                n = nq * W2
                sfx = "b" if nq == 1 else ""
                cs = slice(q0 * W2, (q0 + nq) * W2)
                # f,g first (they feed the serial t1/t2/add chain), then
                # oi as one [128,n] hard-sigmoid-affine ACT (o at 0:64, i at
                # 64:128; clip via min in the gate products below)
                f_t = epool.tile([128, n], dt.float32, tag="f" + sfx)
                nc.scalar.activation(f_t[64:128, :], ps1[64:128, :], AF.Relu,
                                     bias=vsb[64:128, 1:2], scale=0.2)
                g_t = epool.tile([128, n], dt.float32, tag="g" + sfx)
                nc.scalar.activation(g_t[64:128, :], ps1[0:64, :], AF.Tanh,
                                     bias=vsb[0:64, 2:3], scale=1.0)
                io_t = epool.tile([128, n], dt.float32, tag="io" + sfx)
                nc.scalar.activation(io_t[:], ps0, AF.Relu,
                                     bias=vsb[:, 0:1], scale=0.2)
                t1 = epool.tile([128, n], dt.float32, tag="t1" + sfx)
                nc.vector.scalar_tensor_tensor(
                    t1[64:128, :], f_t[64:128, :], 1.0, c_sb[64:128, cs],
                    mybir.AluOpType.min, mybir.AluOpType.mult)
                t2 = epool.tile([128, n], dt.float32, tag="t2" + sfx)
                nc.vector.scalar_tensor_tensor(
                    t2[64:128, :], io_t[64:128, :], 1.0, g_t[64:128, :],
                    mybir.AluOpType.min, mybir.AluOpType.mult)
                nc.vector.tensor_add(c_sb[64:128, cs], t1[64:128, :],
                                     t2[64:128, :])
                return io_t

            def emit_partB(t, q0, nq, io_t, hcr):
                n = nq * W2
                sfx = "b" if nq == 1 else ""
                cs = slice(q0 * W2, (q0 + nq) * W2)
                tc_t = epool.tile([64, n], dt.float32, tag="tc" + sfx)
                nc.scalar.activation(tc_t[:], c_sb[64:128, cs], AF.Tanh)
                hlo = hcr[0:64, q0+1:q0+nq+1, 1:65]
                nc.vector.scalar_tensor_tensor(
                    hlo,
                    io_t[0:64, :].rearrange("p (a b) -> p a b", b=W2), 1.0,
                    tc_t[:].rearrange("p (a b) -> p a b", b=W2),
                    mybir.AluOpType.min, mybir.AluOpType.mult)
                if nq > 1:
                    nc.vector.tensor_copy(
                        out=hcr[64:128, q0:q0+nq, 1:65], in_=hlo)
                yst = ypool.tile([64, n], dt.float32, tag="yst" + sfx)
                yeng = nc.vector if nq == 1 else nc.gpsimd
                yeng.tensor_scalar(
                    yst[:].rearrange("p (a b) -> p a b", b=W2), hlo,
                    vsb[0:64, 3:4], vsb[0:64, 4:5],
                    mybir.AluOpType.mult, mybir.AluOpType.add)
                nc.sync.dma_start(out=y[t, :, q0*W2:(q0+nq)*W2], in_=yst[:])

            for t in range(T):
                hc = h2[t % 2]
                hp = h2[(t + 1) % 2]
                hcr = hc[:].rearrange("p (q w) -> p q w", w=WP)
                hpr = hp[:].rearrange("p (q w) -> p q w", w=WP)

                x3t = xpool.tile([96, R * XW], sdt, tag="x3")
                x3r = x3t[:].rearrange("p (q w) -> p q w", w=XW)
                nc.sync.dma_start(out=x3r[0:32], in_=xs[t, :, 0:2*R-1:2, :])
                nc.sync.dma_start(out=x3r[32:64], in_=xs[t, :, 1:2*R:2, :])
                nc.sync.dma_start(out=x3r[64:96], in_=xs[t, :, 2:2*R+1:2, :])

                # boundary single-row chunk FIRST: its halo input (bout of
                # step t-1) launched at the top of step t-1, so a full step
                # of interior work covers the mesh latency; likewise the
                # exchange launched here is consumed a full step downstream.
                # Its PSUM shares the c3a tiles' spare columns (448+64=512).
                sh = [pspool.tile([128, 512], dt.float32,
                                  name=f"sh_{t}_{mi}", tag="ps")
                      for mi in range(2)]
                pss = emit_mms(t, R - 1, 1, hpr, x3r,
                               pss=[s[:][:, 448:512] for s in sh])
                io_t = emit_partA(t, R - 1, 1, pss)
                emit_partB(t, R - 1, 1, io_t, hcr)

                # halo exchange (skip after last step): bf16; the AllReduce
                # SUM lands directly at q=R+1 parts 0:64 (u2's -U2 rows
                # subtract own inside the boundary matmul). DMAs + trigger on
                # the GPSIMD queue so waits never block ACT/DVE/PE.
                if t < T - 1:
                    bin_d = dpool.tile([64, 64], sdt, tag="bin")
                    bout_d = dpool.tile([64, 64], sdt, tag="bout")
                    nc.gpsimd.dma_start(out=bin_d[:],
                                        in_=hcr[0:64, R, 1:65])
                    nc.gpsimd.collective_compute(
                        "AllReduce", mybir.AluOpType.add,
                        replica_groups=groups,
                        ins=[bin_d[:].opt()], outs=[bout_d[:].opt()])
                    nc.gpsimd.dma_start(out=hcr[0:64, R+1, 1:65],
                                        in_=bout_d[:])

                # interior chunks (c3a third so the boundary row's h[30]
                # input lands mid-step, and every cross-step h edge has
                # >= half a step of slack); partB emitted one chunk behind
                # partA so no engine FIFO waits cross-chunk.
                chunks = [(0, NQ), (NQ, NQ), (3 * NQ, NQ - 1), (2 * NQ, NQ)]
                prevB = None
                for q0, nq in chunks:
                    cpss = [s[:][:, 0:448] for s in sh] \
                        if nq == NQ - 1 else None
                    pss = emit_mms(t, q0, nq, hpr, x3r, pss=cpss)
                    io_t = emit_partA(t, q0, nq, pss)
                    if prevB is not None:
                        emit_partB(t, prevB[0], prevB[1], prevB[2], hcr)
                    prevB = (q0, nq, io_t)
                emit_partB(t, prevB[0], prevB[1], prevB[2], hcr)
    _split_multi_waits(nc, mybir)
    return nc


def _install_ntff_hook():
    """The image's antenv lacks axon_hooks; synthesize it and register the
    ctypes NTFF profile hook so trace=True works under axon."""
    import sys
    import types
    try:
        from antenv.axon_hooks import get_axon_ntff_profile_hook  # noqa
        return
    except ImportError:
        pass
    mod = types.ModuleType("antenv.axon_hooks")
    mod._hook = None

    def set_axon_ntff_profile_hook(h):
        mod._hook = h

    def get_axon_ntff_profile_hook():
        return mod._hook

    mod.set_axon_ntff_profile_hook = set_axon_ntff_profile_hook
    mod.get_axon_ntff_profile_hook = get_axon_ntff_profile_hook
    sys.modules["antenv.axon_hooks"] = mod
    import antenv
    antenv.axon_hooks = mod
    try:
        from trn_agent_boot.trn_boot import _ntff_profile_via_ctypes
        hook = _ntff_profile_via_ctypes("/opt/axon/libaxon_pjrt.so")
        if hook is not None:
            mod._hook = hook
    except Exception:
        pass


def _get_nc():
    key = (MM_DT,)
    if key not in _CACHE:
        _CACHE[key] = _build_nc()
    return _CACHE[key]


def kernel(x, W, U, b, gamma, beta, moving_mean, moving_var):
    from concourse.bass_utils import run_bass_kernel_spmd
    x = np.asarray(x, np.float32)
    W = np.asarray(W, np.float32)
    U = np.asarray(U, np.float32)
    b = np.asarray(b, np.float32)
    gamma = np.asarray(gamma, np.float32)
    beta = np.asarray(beta, np.float32)
    moving_mean = np.asarray(moving_mean, np.float32)
    moving_var = np.asarray(moving_var, np.float32)
    B = x.shape[0]

    in_maps = []
    for bidx in range(B):
        for half in range(N_HALVES):
            in_maps.append(_prep_core_inputs(
                x, W, U, b, gamma, beta, moving_mean, moving_var, bidx, half))

    nc = _get_nc()
    trace = os.environ.get("BASS_KERNEL_TRACE") == "1"
    if trace:
        _install_ntff_hook()
    res = run_bass_kernel_spmd(nc, in_maps, core_ids=list(range(8)),
                               trace=trace)
    kernel._last_result = res

    out = np.zeros((B, T, H2, W2, F), np.float32)
    ci = 0
    for bidx in range(B):
        for half in range(N_HALVES):
            yc = res.results[ci]["y"].reshape(T, F, R, W2)
            ci += 1
            yc = yc.transpose(0, 2, 3, 1)  # (T, R, W2, F)
            if half == 1:
                yc = yc[:, ::-1, :, :]
                out[bidx, :, 32:64] = yc
            else:
                out[bidx, :, 0:32] = yc
    return out
